# revision 24
# baseline (speedup 1.0000x reference)
"""Trainium2 Bass kernel for local (neighbor-list) multi-head attention.

Sharding: 8 cores = 2 frames x 4 atom-chunks (512 local atoms per core).
Per core: project k rows to SBUF (rank-striped) and v rows to DRAM in
fp16, DMA-row-gather neighbors (dma_gather; K from SBUF via transpose
mode, V from HBM; prefetched one block-pair ahead), per-block batched QK
(M=32 block-diag stationaries), softmax over a host-masked full-width
bias (unnormalized; 1/Z folded into the AV-psum evacuation via a
host-provided row-permutation matmul), PE-transpose, paired-atom AV
(M=16 stationaries), diagonal extraction via a DRAM bounce, gating +
output projection decoupled from the main loop.  The PE work is
software-pipelined two blocks deep (QK(b) | transpose(b-1) | AV(b-2))
so the tensor engine never head-blocks on the softmax chain.
"""

import numpy as np

NF, NLOC, NALL, NNEI = 2, 2048, 3072, 128
H, D = 8, 32
TOTAL = H * D          # 256
QDIM = 256
NCORES = 8
CPF = NCORES // NF     # 4 cores per frame
NLOC_C = NLOC // CPF   # 512 atoms per core
BLK = 16               # atoms per block
NBLK = NLOC_C // BLK   # 32
SG = 8                 # blocks per supergroup (=128 atoms)
NSG = NBLK // SG       # 4

_CACHE = {}


def _build():
    import concourse.bass as bass
    import concourse.mybir as mybir
    from concourse import bacc
    from concourse.tile import TileContext
    from concourse.masks import make_identity

    dt = mybir.dt
    f32, f16, i16 = dt.float32, dt.float16, dt.int16
    AF = mybir.ActivationFunctionType

    nc = bacc.Bacc(None, target_bir_lowering=False)

    # ---------------- external inputs (contents differ per core) ------------
    qT = nc.dram_tensor("qT", [QDIM, NLOC_C], f16, kind="ExternalInput")
    kT = nc.dram_tensor("kT", [QDIM, NALL], f16, kind="ExternalInput")
    vT = nc.dram_tensor("vT", [QDIM, NALL], f16, kind="ExternalInput")
    WqT = nc.dram_tensor("WqT", [QDIM, TOTAL], f16, kind="ExternalInput")
    WgT = nc.dram_tensor("WgT", [QDIM, TOTAL], f16, kind="ExternalInput")
    WkT = nc.dram_tensor("WkT", [QDIM, TOTAL], f16, kind="ExternalInput")
    WvT = nc.dram_tensor("WvT", [QDIM, TOTAL], f16, kind="ExternalInput")
    WoTh = nc.dram_tensor("WoTh", [TOTAL, QDIM], f16, kind="ExternalInput")
    bgr = nc.dram_tensor("bgr", [128, TOTAL], f32, kind="ExternalInput")
    bo2 = nc.dram_tensor("bo2", [128, 2], f32, kind="ExternalInput")
    idx = nc.dram_tensor("idx", [128, NBLK * NNEI], i16, kind="ExternalInput")
    bias_p = nc.dram_tensor("bias_p", [NBLK, 128, 4 * NNEI], f16, kind="ExternalInput")
    perm = nc.dram_tensor("perm", [128, 2, 128], f16, kind="ExternalInput")

    out_t = nc.dram_tensor("out_t", [TOTAL, NLOC_C], f32, kind="ExternalOutput")

    kT_r = kT.rearrange("(a p) n -> p a n", p=128)
    vT_r = vT.rearrange("(a p) n -> p a n", p=128)

    with TileContext(nc) as tc:
        with (
            tc.tile_pool(name="const", bufs=1) as const,
            tc.tile_pool(name="work", bufs=2) as work,
            tc.tile_pool(name="gath", bufs=2) as gath,
            tc.tile_pool(name="oph", bufs=1) as oph,
            tc.tile_pool(name="psQK", bufs=2, space="PSUM") as psQK,
            tc.tile_pool(name="psPT", bufs=2, space="PSUM") as psPT,
            tc.tile_pool(name="psAV", bufs=3, space="PSUM") as psAV,
            tc.tile_pool(name="psO", bufs=1, space="PSUM") as psO,
            tc.tile_pool(name="dram", bufs=1, space="DRAM") as dram,
        ):
            # ---------------- constants -------------------------------------
            ident = const.tile([128, 128], f16, tag="ident")
            make_identity(nc, ident)

            idx_tiles = {}

            def load_idx(sg):
                idx_t = work.tile([128, SG * NNEI], i16, tag="idx_t")
                nc.sync.dma_start(
                    idx_t, idx[:, SG * NNEI * sg:SG * NNEI * (sg + 1)]
                )
                idx_tiles[sg] = idx_t

            load_idx(0)
            wk = const.tile([128, 2, TOTAL], f16, tag="wk")
            nc.sync.dma_start(wk, WkT.rearrange("(a p) o -> p a o", p=128))
            wv = const.tile([128, 2, TOTAL], f16, tag="wv")
            nc.sync.dma_start(wv, WvT.rearrange("(a p) o -> p a o", p=128))

            # ---------------- K table (SBUF, rank-striped) -------------------
            khs = const.tile([128, NALL // 128, TOTAL], f16, tag="khs")
            for jc4 in range(NALL // 512):
                kTc = work.tile([128, 2, 512], f16, tag="kTc", bufs=1)
                nc.sync.dma_start(kTc, kT_r[:, :, 512 * jc4:512 * (jc4 + 1)])
                for j4 in range(4):
                    jc = 4 * jc4 + j4
                    ps = psQK.tile([128, TOTAL], f32, tag="qk", name="ps_k")
                    for cc in range(2):
                        nc.tensor.matmul(
                            ps, kTc[:, cc, 128 * j4:128 * (j4 + 1)], wk[:, cc, :],
                            start=(cc == 0), stop=(cc == 1),
                        )
                    nc.scalar.copy(khs[:, jc, :], ps)

            # ---------------- V table (DRAM rows, fp16) ----------------------
            vh_d = dram.tile([NALL, TOTAL], f16)
            for jc4 in range(NALL // 512):
                vTc = work.tile([128, 2, 512], f16, tag="kTc", bufs=1)
                nc.sync.dma_start(vTc, vT_r[:, :, 512 * jc4:512 * (jc4 + 1)])
                row16 = work.tile([128, 4, TOTAL], f16, tag="row16", bufs=1)
                for j4 in range(4):
                    ps = psQK.tile([128, TOTAL], f32, tag="qk", name="ps_v")
                    for cc in range(2):
                        nc.tensor.matmul(
                            ps, vTc[:, cc, 128 * j4:128 * (j4 + 1)], wv[:, cc, :],
                            start=(cc == 0), stop=(cc == 1),
                        )
                    nc.scalar.copy(row16[:, j4, :], ps)
                nc.scalar.dma_start(
                    vh_d[512 * jc4:512 * (jc4 + 1), :].rearrange(
                        "(c p) o -> p c o", p=128
                    ),
                    row16,
                )

            # ---------------- gather issue (prefetched one pair ahead) -------
            gath_tiles = {}

            def issue_gathers(pair):
                b0 = 2 * pair
                idx_sl = idx_tiles[b0 // SG][:, NNEI * (b0 % SG):NNEI * (b0 % SG + 2)]
                kgT = gath.tile([128, 2, 2 * BLK * NNEI], f16, tag="kgT")
                nc.gpsimd.dma_gather(
                    kgT, khs[:, :, :], idx_sl,
                    num_idxs=2 * BLK * NNEI, num_idxs_reg=2 * BLK * NNEI,
                    elem_size=TOTAL, transpose=True, queue_num=0,
                    single_packet=False,
                    sbuf_tokens_per_rank=128,
                    sbuf_free_dim_per_rank=2 * TOTAL,
                    sbuf_free_dim_pad_per_rank=0,
                    sbuf_byte_offset=0,
                )
                vg = gath.tile([128, 2 * BLK, TOTAL], f16, tag="vg", bufs=2)
                nc.gpsimd.dma_gather(
                    vg, vh_d[:, :], idx_sl,
                    num_idxs=2 * BLK * NNEI, num_idxs_reg=2 * BLK * NNEI,
                    elem_size=TOTAL, transpose=False, queue_num=0,
                    single_packet=False,
                )
                gath_tiles[pair] = (kgT, vg)

            issue_gathers(0)   # K side ready as soon as khs lands
            _PREFETCH = False

            # ---------------- per-supergroup bias ----------------------------
            bias_tiles = {}

            def load_bias(sg):
                bias_t = work.tile([128, SG, 4 * NNEI], f16, tag="bias_t", bufs=1)
                nc.sync.dma_start(
                    bias_t, bias_p[SG * sg:SG * (sg + 1)].rearrange("b p i -> p b i")
                )
                bias_tiles[sg] = bias_t

            load_bias(0)

            # ---------------- q-side ------------------------------------------
            wq = const.tile([128, 2, TOTAL], f16, tag="wq")
            nc.sync.dma_start(wq, WqT.rearrange("(a p) o -> p a o", p=128))
            wg = const.tile([128, 2, TOTAL], f16, tag="wg")
            nc.sync.dma_start(wg, WgT.rearrange("(a p) o -> p a o", p=128))
            wo = const.tile([128, 2, QDIM], f16, tag="wo")
            nc.sync.dma_start(wo, WoTh.rearrange("(a p) o -> p a o", p=128))
            bg_t = const.tile([128, TOTAL], f32, tag="bg_t")
            nc.sync.dma_start(bg_t, bgr[:, :])
            bo_t = const.tile([128, 2], f32, tag="bo_t")
            nc.sync.dma_start(bo_t, bo2[:, :])
            qT_t = const.tile([128, 2, NLOC_C], f16, tag="qT_t")
            nc.sync.dma_start(qT_t, qT.rearrange("(a p) n -> p a n", p=128))
            perm8 = const.tile([128, 2, 128], f16, tag="perm8")
            nc.sync.dma_start(perm8, perm[:, :, :])



            # qhT (fp16, [hd_chunk][128, NLOC_C])
            qhT = const.tile([128, 2, NLOC_C], f16, tag="qhT")
            for hc in range(2):
                ps = psQK.tile([128, NLOC_C], f32, tag="qk", name="ps_qh")
                for cc in range(2):
                    nc.tensor.matmul(
                        ps, wq[:, cc, 128 * hc:128 * (hc + 1)], qT_t[:, cc, :],
                        start=(cc == 0), stop=(cc == 1),
                    )
                nc.scalar.copy(qhT[:, hc, :], ps)

            # sigmoid(g) rows: [n_chunk][128, 256]
            sig_g = const.tile([128, 4, TOTAL], f32, tag="sig_g")
            for ncnk in range(4):
                ps = psQK.tile([128, TOTAL], f32, tag="qk", name="ps_g")
                for cc in range(2):
                    nc.tensor.matmul(
                        ps, qT_t[:, cc, 128 * ncnk:128 * (ncnk + 1)], wg[:, cc, :],
                        start=(cc == 0), stop=(cc == 1),
                    )
                gtmp = work.tile([128, TOTAL], f32, tag="gtmp", bufs=1)
                nc.vector.tensor_add(gtmp, ps, bg_t)
                nc.scalar.activation(sig_g[:, ncnk, :], gtmp, AF.Sigmoid)

            # qblk: block-diagonal stationaries [128, ch, NBLK*4 groups * 32]
            qblk = const.tile([128, 2, (NLOC_C // 4) * 32], f16, tag="qblk")
            nc.gpsimd.memset(qblk, 0.0)
            for ch in range(2):
                for qq in range(4):
                    h = 4 * ch + qq
                    dst = qblk[32 * qq:32 * (qq + 1), ch, :].rearrange(
                        "p (G c) -> p G c", c=32
                    )[:, :, 4 * h:4 * h + 4]
                    src = qhT[32 * qq:32 * (qq + 1), ch, :].rearrange(
                        "p (G a) -> p G a", a=4
                    )
                    nc.vector.tensor_copy(dst, src)

            # staging tensors
            o_scr = dram.tile([NLOC_C, TOTAL], f16)
            o_r = o_scr.rearrange(
                "(sg blk p01 g01 asub) (h d) -> sg asub blk p01 g01 h d",
                sg=NSG, blk=SG, p01=2, g01=2, asub=4, h=H,
            )

            # ---------------- software-pipelined main loop --------------------
            # stage A (block b):   QK + softmax chain + 1/Z recip
            # stage B (block b-1): P transposes + 1/Z permutation matmuls
            # stage C (block b-2): AV + scaled evac (+ extract cadence)
            st = {}            # per-block tiles
            stage = None
            pending = None     # (sg, orow) or (sg, orow, god)
            for it in range(NBLK + 3):
                b = it
                if b < NBLK:
                    if b % 2 == 0:
                        if b > 0:
                            issue_gathers(b // 2)
                        if b % SG == 0 and b + SG < NBLK:
                            load_bias(b // SG + 1)
                            load_idx(b // SG + 1)
                    kgT = gath_tiles[b // 2][0]
                    qk = psQK.tile([128, 4 * NNEI], f32, tag="qk", name="qk")
                    for g in range(4):
                        for cc in range(2):
                            nc.tensor.matmul(
                                qk[32 * g:32 * (g + 1), :],
                                qblk[:, cc, 32 * (4 * b + g):32 * (4 * b + g + 1)],
                                kgT[:, cc, 512 * (4 * (b % 2) + g):512 * (4 * (b % 2) + g + 1)],
                                start=(cc == 0), stop=(cc == 1),
                                tile_position=(0, 32 * g),
                            )
                    # 1/Z for the previous block: first in the DVE queue this
                    # iteration so the stage-B permutation matmuls never stall
                    if b - 1 >= 0:
                        Zi_b = work.tile([128, 1], f16, tag="Zi_b", bufs=3)
                        with nc.allow_low_precision(reason="1/Z feeds fp16 p"):
                            nc.vector.reciprocal(Zi_b, st[b - 1]["Zb"])
                        st[b - 1]["Zi_b"] = Zi_b
                    s_t = work.tile([128, 4 * NNEI], f32, tag="s_t", bufs=2)
                    nc.vector.tensor_add(s_t, qk, bias_tiles[b // SG][:, b % SG, :])
                    m_t = work.tile([128, 1], f32, tag="m_t", bufs=3)
                    nc.vector.reduce_max(
                        m_t, s_t, axis=mybir.AxisListType.X, negate=True
                    )
                    p_t = work.tile([128, 4 * NNEI], f16, tag="p_t", bufs=3)
                    Zb = work.tile([128, 1], f32, tag="Zb", bufs=3)
                    nc.scalar.activation(
                        p_t, s_t, AF.Exp, bias=m_t, scale=1.0, accum_out=Zb,
                    )
                    st[b] = {"p_t": p_t, "Zb": Zb}

                # ---- stage B: block b-2 ----
                if 0 <= b - 2 < NBLK:
                    sb = st[b - 2]
                    pt_ps = psPT.tile([128, 4 * NNEI], f16, tag="pt")
                    for j in range(4):
                        nc.tensor.transpose(
                            pt_ps[:, 128 * j:128 * (j + 1)],
                            sb["p_t"][:, 128 * j:128 * (j + 1)], ident,
                        )
                    if "Zi_b" not in sb:   # last block: stage A already ended
                        Zi_b = work.tile([128, 1], f16, tag="Zi_b", bufs=3)
                        with nc.allow_low_precision(reason="1/Z feeds fp16 p"):
                            nc.vector.reciprocal(Zi_b, sb["Zb"])
                        sb["Zi_b"] = Zi_b
                    Zi_b = sb["Zi_b"]
                    zp_ps = psO.tile([128, 2], f32, tag="o", name="zp_ps")
                    for p01 in range(2):
                        nc.tensor.matmul(
                            zp_ps[:, p01:p01 + 1], perm8[:, p01, :], Zi_b,
                            start=True, stop=True,
                        )
                    pT = work.tile([128, 4, 128], f16, tag="pT", bufs=3)
                    nc.vector.tensor_copy(pT.rearrange("p w c -> p (w c)"), pt_ps)
                    ZiPs = work.tile([128, 2], f32, tag="ZiPs", bufs=3)
                    nc.vector.tensor_copy(ZiPs, zp_ps)
                    sb["pT"] = pT
                    sb["ZiPs"] = ZiPs

                # ---- output phase, part 2: gating (uses orow readback) ----
                if pending is not None and len(pending) == 2 and b % SG == 5:
                    sg, orow = pending
                    god = oph.tile([128, TOTAL], f16, tag="god")
                    nc.vector.tensor_mul(god, orow, sig_g[:, sg, :])
                    pending = (sg, orow, god)

                # ---- stage C: block b-3 ----
                if 0 <= b - 3 < NBLK:
                    bb = b - 3
                    sc = st.pop(bb)
                    vg = gath_tiles[bb // 2][1]
                    pT_r = sc["pT"].rearrange(
                        "p w (pp g h a) -> p w pp g h a", pp=2, g=2, h=H, a=4
                    )
                    av0 = psAV.tile([128, 512], f32, tag="av", name="av0")
                    av1 = psAV.tile([128, 512], f32, tag="av", name="av1")
                    avs = (av0, av1)
                    for p01 in range(2):
                        for asub in range(4):
                            s0 = 16 * (bb % 2) + 8 * p01 + asub
                            nc.tensor.matmul(
                                avs[p01][32 * asub:32 * asub + 16, :],
                                pT_r[:, asub, p01, :, :, asub],
                                vg[:, s0:s0 + 5:4, :],
                                start=True, stop=True,
                                tile_position=(0, 32 * asub),
                            )
                    if bb % SG == 0:
                        stage = work.tile([128, SG * 1024], f16, tag="stage")
                    nc.vector.tensor_scalar_mul(
                        stage[:, 1024 * (bb % SG):1024 * (bb % SG) + 512], av0,
                        sc["ZiPs"][:, 0:1],
                    )
                    nc.scalar.activation(
                        stage[:, 1024 * (bb % SG) + 512:1024 * (bb % SG + 1)],
                        av1, AF.Identity, scale=sc["ZiPs"][:, 1:2],
                    )

                    if bb % SG == SG - 1:
                        sg = bb // SG
                        st_r = stage.rearrange(
                            "p (blk p01 g01 h d) -> p blk p01 g01 h d",
                            blk=SG, p01=2, g01=2, h=H,
                        )
                        eng = (nc.sync, nc.scalar, nc.gpsimd)
                        for g01 in range(2):
                            for h in range(H):
                                eng[(g01 * H + h) % 3].dma_start(
                                    o_r[sg, :, :, :, g01, h, :],
                                    st_r[8 * g01 + h::32, :, :, g01, h, :],
                                )
                        orow = oph.tile([128, TOTAL], f16, tag="orow")
                        nc.gpsimd.dma_start(
                            orow, o_scr[128 * sg:128 * (sg + 1), :]
                        )
                        pending = (sg, orow)

                # ---- output phase, part 3: projection + store ----
                if pending is not None and len(pending) == 3 and b % SG == 7:
                    sg, orow, god = pending
                    godT = oph.tile([128, 2, 128], f16, tag="godT")
                    for hc in range(2):
                        gps = psO.tile([128, 128], f16, tag="o", name="gps")
                        nc.tensor.transpose(
                            gps, god[:, 128 * hc:128 * (hc + 1)], ident
                        )
                        nc.scalar.copy(godT[:, hc, :], gps)
                    for oc in range(2):
                        ops = psO.tile([128, 128], f32, tag="o", name="ops")
                        for hc in range(2):
                            nc.tensor.matmul(
                                ops, wo[:, hc, 128 * oc:128 * (oc + 1)],
                                godT[:, hc, :],
                                start=(hc == 0), stop=(hc == 1),
                            )
                        outs = oph.tile([128, 128], f32, tag="outs")
                        nc.scalar.activation(
                            outs, ops, AF.Identity, bias=bo_t[:, oc:oc + 1]
                        )
                        nc.scalar.dma_start(
                            out_t[128 * oc:128 * (oc + 1), 128 * sg:128 * (sg + 1)],
                            outs,
                        )
                    pending = None

            # drain the last supergroup's output phase
            if pending is not None:
                sg, orow = pending[0], pending[1]
                god = oph.tile([128, TOTAL], f16, tag="god")
                nc.vector.tensor_mul(god, orow, sig_g[:, sg, :])
                godT = oph.tile([128, 2, 128], f16, tag="godT")
                for hc in range(2):
                    gps = psO.tile([128, 128], f16, tag="o", name="gps")
                    nc.tensor.transpose(
                        gps, god[:, 128 * hc:128 * (hc + 1)], ident
                    )
                    nc.scalar.copy(godT[:, hc, :], gps)
                for oc in range(2):
                    ops = psO.tile([128, 128], f32, tag="o", name="ops")
                    for hc in range(2):
                        nc.tensor.matmul(
                            ops, wo[:, hc, 128 * oc:128 * (oc + 1)],
                            godT[:, hc, :],
                            start=(hc == 0), stop=(hc == 1),
                        )
                    outs = oph.tile([128, 128], f32, tag="outs")
                    nc.scalar.activation(
                        outs, ops, AF.Identity, bias=bo_t[:, oc:oc + 1]
                    )
                    nc.scalar.dma_start(
                        out_t[128 * oc:128 * (oc + 1), 128 * sg:128 * (sg + 1)],
                        outs,
                    )
    nc.finalize()
    return nc


def _host_prep(q, k, v, nlist, bias, Wq, Wk, Wv, Wg, bg, Wo, bo):
    """Build the 8 per-core input maps."""
    norm = D ** -0.5
    f32 = np.float32
    WqT = np.ascontiguousarray((Wq * norm).T.astype(np.float16))
    WgT = np.ascontiguousarray(Wg.T.astype(np.float16))
    WkT = np.ascontiguousarray(Wk.T.astype(np.float16))
    WvT = np.ascontiguousarray(Wv.T.astype(np.float16))
    WoTh = np.ascontiguousarray(Wo.T.astype(np.float16))
    bgr = np.ascontiguousarray(np.broadcast_to(bg.astype(f32), (128, TOTAL)))
    bo2 = np.ascontiguousarray(bo.astype(f32).reshape(2, 128).T)
    # perm[rz, p01, rav] = 1 iff rz = 64*p01 + 32*g01 + 4*h + asub
    # for rav = 32*asub + 8*g01 + h  (AV-psum row <- softmax row Z source)
    perm = np.zeros((128, 2, 128), np.float16)
    for p01 in range(2):
        for asub in range(4):
            for g01 in range(2):
                for h in range(H):
                    rav = 32 * asub + 8 * g01 + h
                    rz = 64 * p01 + 32 * g01 + 4 * h + asub
                    perm[rz, p01, rav] = 1.0

    in_maps = []
    for c in range(NCORES):
        f, chunk = c // CPF, c % CPF
        n0 = chunk * NLOC_C
        qc = q[f, n0:n0 + NLOC_C]                     # [512, 256]
        nl = nlist[f, n0:n0 + NLOC_C].astype(np.int16)  # [512, 128]
        # wrapped gather indices: per block b, t-th index at [16g + t%16, t//16]
        w = nl.reshape(NBLK, BLK * NNEI).reshape(NBLK, BLK * NNEI // 16, 16)
        w = np.transpose(w, (0, 2, 1)).reshape(NBLK, 16, -1)   # [b, 16, 128]
        w = np.concatenate([w] * 8, axis=1)                    # [b, 128, 128]
        idx_full = np.ascontiguousarray(
            np.transpose(w, (1, 0, 2)).reshape(128, NBLK * NNEI)
        )
        # bias: [8, 512, 128] -> [32 blocks, (g h asub), 128]
        bs = bias[f, :, n0:n0 + NLOC_C, :]
        from einops import rearrange as rr
        bias_cmp = rr(bs, "h (b g asub) i -> b (g h asub) i", b=NBLK, g=4, asub=4)
        bias_c = np.full((NBLK, 128, 4 * NNEI), -30000.0, np.float16)
        p_arange = np.arange(128)
        for asub in range(4):
            rows = p_arange[p_arange % 4 == asub]
            bias_c[:, rows, NNEI * asub:NNEI * (asub + 1)] = (
                bias_cmp[:, rows, :].astype(np.float16)
            )
        in_maps.append({
            "qT": np.ascontiguousarray(qc.T.astype(np.float16)),
            "kT": np.ascontiguousarray(k[f].T.astype(np.float16)),
            "vT": np.ascontiguousarray(v[f].T.astype(np.float16)),
            "WqT": WqT, "WgT": WgT, "WkT": WkT, "WvT": WvT, "WoTh": WoTh,
            "bgr": bgr, "bo2": bo2,
            "idx": idx_full, "bias_p": bias_c, "perm": perm,
        })
    return in_maps


def kernel(q, k, v, nlist, bias, Wq, Wk, Wv, Wg, bg, Wo, bo):
    from concourse.bass_utils import run_bass_kernel_spmd

    q = np.asarray(q, dtype=np.float32)
    k = np.asarray(k, dtype=np.float32)
    v = np.asarray(v, dtype=np.float32)
    bias = np.asarray(bias, dtype=np.float32)
    nlist_np = np.asarray(nlist)

    if "nc" not in _CACHE:
        _CACHE["nc"] = _build()
    nc = _CACHE["nc"]

    in_maps = _host_prep(
        q, k, v, nlist_np, bias,
        np.asarray(Wq, np.float32), np.asarray(Wk, np.float32),
        np.asarray(Wv, np.float32), np.asarray(Wg, np.float32),
        np.asarray(bg, np.float32), np.asarray(Wo, np.float32),
        np.asarray(bo, np.float32),
    )
    res = run_bass_kernel_spmd(nc, in_maps, core_ids=list(range(NCORES)))
    out = np.empty((NF, NLOC, TOTAL), dtype=np.float32)
    for c in range(NCORES):
        f, chunk = c // CPF, c % CPF
        n0 = chunk * NLOC_C
        out[f, n0:n0 + NLOC_C] = res.results[c]["out_t"].T
    return out


# revision 41
# speedup vs baseline: 1.0525x; 1.0525x over previous
"""Trainium2 Bass kernel for local (neighbor-list) multi-head attention.

Sharding: 8 cores = 2 frames x 4 atom-chunks (512 local atoms per core).
Per core: project k rows to SBUF (rank-striped) and v rows to DRAM in
fp16, DMA-row-gather neighbors (dma_gather; K from SBUF via transpose
mode, V from HBM; prefetched one block-pair ahead), per-block batched QK
(M=32 block-diag stationaries), softmax over a host-masked full-width
bias (unnormalized; 1/Z folded into the AV-psum evacuation via a
host-provided row-permutation matmul), PE-transpose, paired-atom AV
(M=16 stationaries), diagonal extraction via a DRAM bounce, gating +
output projection decoupled from the main loop.  The PE work is
software-pipelined three blocks deep (QK(b) | transpose(b-2) | AV(b-3))
so the tensor engine never head-blocks on the softmax chain.
"""

import numpy as np

NF, NLOC, NALL, NNEI = 2, 2048, 3072, 128
H, D = 8, 32
TOTAL = H * D          # 256
QDIM = 256
NCORES = 8
CPF = NCORES // NF     # 4 cores per frame
NLOC_C = NLOC // CPF   # 512 atoms per core
BLK = 16               # atoms per block
NBLK = NLOC_C // BLK   # 32
SG = 8                 # blocks per supergroup (=128 atoms)
NSG = NBLK // SG       # 4

_CACHE = {}


def _build():
    import concourse.bass as bass
    import concourse.mybir as mybir
    from concourse import bacc
    from concourse.tile import TileContext
    from concourse.masks import make_identity

    dt = mybir.dt
    f32, f16, i16 = dt.float32, dt.float16, dt.int16
    AF = mybir.ActivationFunctionType

    nc = bacc.Bacc(None, target_bir_lowering=False)

    # ---------------- external inputs (contents differ per core) ------------
    qT = nc.dram_tensor("qT", [QDIM, NLOC_C], f16, kind="ExternalInput")
    kT = nc.dram_tensor("kT", [QDIM, NALL], f16, kind="ExternalInput")
    vT = nc.dram_tensor("vT", [QDIM, NALL], f16, kind="ExternalInput")
    WqT = nc.dram_tensor("WqT", [QDIM, TOTAL], f16, kind="ExternalInput")
    WgT = nc.dram_tensor("WgT", [QDIM, TOTAL], f16, kind="ExternalInput")
    WkT = nc.dram_tensor("WkT", [QDIM, TOTAL], f16, kind="ExternalInput")
    WvT = nc.dram_tensor("WvT", [QDIM, TOTAL], f16, kind="ExternalInput")
    WoTh = nc.dram_tensor("WoTh", [TOTAL, QDIM], f16, kind="ExternalInput")
    bgr = nc.dram_tensor("bgr", [128, TOTAL], f32, kind="ExternalInput")
    bo2 = nc.dram_tensor("bo2", [128, 2], f32, kind="ExternalInput")
    idx = nc.dram_tensor("idx", [128, NBLK * NNEI], i16, kind="ExternalInput")
    bias_p = nc.dram_tensor("bias_p", [NBLK, 128, 4 * NNEI], f16, kind="ExternalInput")
    perm = nc.dram_tensor("perm", [128, 2, 128], f16, kind="ExternalInput")

    out_t = nc.dram_tensor("out_t", [TOTAL, NLOC_C], f32, kind="ExternalOutput")

    kT_r = kT.rearrange("(a p) n -> p a n", p=128)
    vT_r = vT.rearrange("(a p) n -> p a n", p=128)

    with TileContext(nc) as tc:
        with (
            tc.tile_pool(name="const", bufs=1) as const,
            tc.tile_pool(name="work", bufs=2) as work,
            tc.tile_pool(name="gath", bufs=2) as gath,
            tc.tile_pool(name="oph", bufs=2) as oph,
            tc.tile_pool(name="psQK", bufs=2, space="PSUM") as psQK,
            tc.tile_pool(name="psPT", bufs=2, space="PSUM") as psPT,
            tc.tile_pool(name="psAV", bufs=3, space="PSUM") as psAV,
            tc.tile_pool(name="psO", bufs=1, space="PSUM") as psO,
            tc.tile_pool(name="dram", bufs=1, space="DRAM") as dram,
        ):
            # ---------------- constants -------------------------------------
            ident = const.tile([128, 128], f16, tag="ident")
            make_identity(nc, ident)

            idx_tiles = {}

            def load_idx(sg):
                idx_t = work.tile([128, SG * NNEI], i16, tag="idx_t")
                nc.sync.dma_start(
                    idx_t, idx[:, SG * NNEI * sg:SG * NNEI * (sg + 1)]
                )
                idx_tiles[sg] = idx_t

            load_idx(0)
            wk = const.tile([128, 2, TOTAL], f16, tag="wk")
            nc.sync.dma_start(wk, WkT.rearrange("(a p) o -> p a o", p=128))
            wv = const.tile([128, 2, TOTAL], f16, tag="wv")
            nc.sync.dma_start(wv, WvT.rearrange("(a p) o -> p a o", p=128))

            # ---------------- K table (SBUF, rank-striped) -------------------
            khs = const.tile([128, NALL // 128, TOTAL], f16, tag="khs")
            for jc4 in range(NALL // 512):
                kTc = work.tile([128, 2, 512], f16, tag="kTc", bufs=1)
                nc.sync.dma_start(kTc, kT_r[:, :, 512 * jc4:512 * (jc4 + 1)])
                for j4 in range(4):
                    jc = 4 * jc4 + j4
                    ps = psQK.tile([128, TOTAL], f32, tag="qk", name="ps_k")
                    for cc in range(2):
                        nc.tensor.matmul(
                            ps, kTc[:, cc, 128 * j4:128 * (j4 + 1)], wk[:, cc, :],
                            start=(cc == 0), stop=(cc == 1),
                        )
                    nc.scalar.copy(khs[:, jc, :], ps)

            # ---------------- V table (DRAM rows, fp16) ----------------------
            vh_d = dram.tile([NALL, TOTAL], f16)
            for jc4 in range(NALL // 512):
                vTc = work.tile([128, 2, 512], f16, tag="kTc", bufs=1)
                nc.sync.dma_start(vTc, vT_r[:, :, 512 * jc4:512 * (jc4 + 1)])
                row16 = work.tile([128, 4, TOTAL], f16, tag="row16", bufs=1)
                for j4 in range(4):
                    ps = psQK.tile([128, TOTAL], f32, tag="qk", name="ps_v")
                    for cc in range(2):
                        nc.tensor.matmul(
                            ps, vTc[:, cc, 128 * j4:128 * (j4 + 1)], wv[:, cc, :],
                            start=(cc == 0), stop=(cc == 1),
                        )
                    nc.vector.tensor_copy(row16[:, j4, :], ps)
                nc.scalar.dma_start(
                    vh_d[512 * jc4:512 * (jc4 + 1), :].rearrange(
                        "(c p) o -> p c o", p=128
                    ),
                    row16,
                )

            # ---------------- gather issue (prefetched one pair ahead) -------
            gath_tiles = {}

            def issue_gathers(pair):
                b0 = 2 * pair
                idx_sl = idx_tiles[b0 // SG][:, NNEI * (b0 % SG):NNEI * (b0 % SG + 2)]
                kgT = gath.tile([128, 2, 2 * BLK * NNEI], f16, tag="kgT", bufs=3)
                nc.gpsimd.dma_gather(
                    kgT, khs[:, :, :], idx_sl,
                    num_idxs=2 * BLK * NNEI, num_idxs_reg=2 * BLK * NNEI,
                    elem_size=TOTAL, transpose=True, queue_num=0,
                    single_packet=False,
                    sbuf_tokens_per_rank=128,
                    sbuf_free_dim_per_rank=2 * TOTAL,
                    sbuf_free_dim_pad_per_rank=0,
                    sbuf_byte_offset=0,
                )
                vg = gath.tile([128, 2 * BLK, TOTAL], f16, tag="vg", bufs=2)
                nc.gpsimd.dma_gather(
                    vg, vh_d[:, :], idx_sl,
                    num_idxs=2 * BLK * NNEI, num_idxs_reg=2 * BLK * NNEI,
                    elem_size=TOTAL, transpose=False, queue_num=0,
                    single_packet=False,
                )
                gath_tiles[pair] = (kgT, vg)

            issue_gathers(0)   # K side ready as soon as khs lands

            # ---------------- per-supergroup bias ----------------------------
            bias_tiles = {}

            def load_bias(sg):
                bias_t = work.tile([128, SG, 4 * NNEI], f16, tag="bias_t", bufs=1)
                nc.sync.dma_start(
                    bias_t, bias_p[SG * sg:SG * (sg + 1)].rearrange("b p i -> p b i")
                )
                bias_tiles[sg] = bias_t

            load_bias(0)

            # ---------------- q-side ------------------------------------------
            wq = const.tile([128, 2, TOTAL], f16, tag="wq")
            nc.sync.dma_start(wq, WqT.rearrange("(a p) o -> p a o", p=128))
            wg = const.tile([128, 2, TOTAL], f16, tag="wg")
            nc.sync.dma_start(wg, WgT.rearrange("(a p) o -> p a o", p=128))
            wo = const.tile([128, 2, QDIM], f16, tag="wo")
            nc.sync.dma_start(wo, WoTh.rearrange("(a p) o -> p a o", p=128))
            bg_t = const.tile([128, TOTAL], f32, tag="bg_t")
            nc.sync.dma_start(bg_t, bgr[:, :])
            bo_t = const.tile([128, 2], f32, tag="bo_t")
            nc.sync.dma_start(bo_t, bo2[:, :])
            qT_t = const.tile([128, 2, NLOC_C], f16, tag="qT_t")
            nc.sync.dma_start(qT_t, qT.rearrange("(a p) n -> p a n", p=128))
            perm8 = const.tile([128, 2, 128], f16, tag="perm8")
            nc.sync.dma_start(perm8, perm[:, :, :])



            # qhT (fp16, [hd_chunk][128, NLOC_C])
            qhT = const.tile([128, 2, NLOC_C], f16, tag="qhT")
            for hc in range(2):
                ps = psQK.tile([128, NLOC_C], f32, tag="qk", name="ps_qh")
                for cc in range(2):
                    nc.tensor.matmul(
                        ps, wq[:, cc, 128 * hc:128 * (hc + 1)], qT_t[:, cc, :],
                        start=(cc == 0), stop=(cc == 1),
                    )
                nc.scalar.copy(qhT[:, hc, :], ps)

            # sigmoid(g) rows: [n_chunk][128, 256]
            sig_g = const.tile([128, 4, TOTAL], f32, tag="sig_g")
            for ncnk in range(4):
                ps = psQK.tile([128, TOTAL], f32, tag="qk", name="ps_g")
                for cc in range(2):
                    nc.tensor.matmul(
                        ps, qT_t[:, cc, 128 * ncnk:128 * (ncnk + 1)], wg[:, cc, :],
                        start=(cc == 0), stop=(cc == 1),
                    )
                gtmp = work.tile([128, TOTAL], f32, tag="gtmp", bufs=1)
                nc.vector.tensor_add(gtmp, ps, bg_t)
                nc.scalar.activation(sig_g[:, ncnk, :], gtmp, AF.Sigmoid)

            # qblk: block-diagonal stationaries [128, ch, NBLK*4 groups * 32]
            qblk = const.tile([128, 2, (NLOC_C // 4) * 32], f16, tag="qblk")
            nc.gpsimd.memset(qblk, 0.0)
            for ch in range(2):
                for qq in range(4):
                    h = 4 * ch + qq
                    dst = qblk[32 * qq:32 * (qq + 1), ch, :].rearrange(
                        "p (G c) -> p G c", c=32
                    )[:, :, 4 * h:4 * h + 4]
                    src = qhT[32 * qq:32 * (qq + 1), ch, :].rearrange(
                        "p (G a) -> p G a", a=4
                    )
                    nc.vector.tensor_copy(dst, src)

            # staging tensors
            o_scr = dram.tile([NLOC_C, TOTAL], f16)
            o_r = o_scr.rearrange(
                "(sg blk p01 g01 asub) (h d) -> sg asub blk p01 g01 h d",
                sg=NSG, blk=SG, p01=2, g01=2, asub=4, h=H,
            )

            # ---------------- software-pipelined main loop --------------------
            # stage A (block b):   QK + softmax chain + 1/Z recip
            # stage B (block b-1): P transposes + 1/Z permutation matmuls
            # stage C (block b-2): AV + scaled evac (+ extract cadence)
            st = {}            # per-block tiles
            stage = None
            pending = None     # (sg, orow) or (sg, orow, god)
            for it in range(NBLK + 3):
                b = it
                if b < NBLK:
                    if b % 2 == 0:
                        if b > 0:
                            issue_gathers(b // 2)
                        if b % SG == 0 and b + SG < NBLK:
                            load_bias(b // SG + 1)
                            load_idx(b // SG + 1)
                    kgT = gath_tiles[b // 2][0]
                    qk = psQK.tile([128, 4 * NNEI], f32, tag="qk", name="qk")
                    for g in range(4):
                        for cc in range(2):
                            nc.tensor.matmul(
                                qk[32 * g:32 * (g + 1), :],
                                qblk[:, cc, 32 * (4 * b + g):32 * (4 * b + g + 1)],
                                kgT[:, cc, 512 * (4 * (b % 2) + g):512 * (4 * (b % 2) + g + 1)],
                                start=(cc == 0), stop=(cc == 1),
                                tile_position=(0, 32 * g),
                            )
                    # 1/Z for the previous block: first in the DVE queue this
                    # iteration so the stage-B permutation matmuls never stall
                    if b - 1 >= 0:
                        Zi_b = work.tile([128, 1], f16, tag="Zi_b", bufs=4)
                        with nc.allow_low_precision(reason="1/Z feeds fp16 p"):
                            nc.vector.reciprocal(Zi_b, st[b - 1]["Zb"])
                        st[b - 1]["Zi_b"] = Zi_b
                    s_t = work.tile([128, 4 * NNEI], f32, tag="s_t", bufs=2)
                    nc.vector.tensor_add(s_t, qk, bias_tiles[b // SG][:, b % SG, :])
                    m_t = work.tile([128, 1], f32, tag="m_t", bufs=4)
                    nc.vector.reduce_max(
                        m_t, s_t, axis=mybir.AxisListType.X, negate=True
                    )
                    p_t = work.tile([128, 4 * NNEI], f16, tag="p_t", bufs=3)
                    Zb = work.tile([128, 1], f32, tag="Zb", bufs=4)
                    nc.scalar.activation(
                        p_t, s_t, AF.Exp, bias=m_t, scale=1.0, accum_out=Zb,
                    )
                    st[b] = {"p_t": p_t, "Zb": Zb}

                # ---- stage B: block b-2 ----
                if 0 <= b - 2 < NBLK:
                    sb = st[b - 2]
                    pt_ps = psPT.tile([128, 4 * NNEI], f16, tag="pt")
                    for j in range(4):
                        nc.tensor.transpose(
                            pt_ps[:, 128 * j:128 * (j + 1)],
                            sb["p_t"][:, 128 * j:128 * (j + 1)], ident,
                        )
                    if "Zi_b" not in sb:   # last block: stage A already ended
                        Zi_b = work.tile([128, 1], f16, tag="Zi_b", bufs=4)
                        with nc.allow_low_precision(reason="1/Z feeds fp16 p"):
                            nc.vector.reciprocal(Zi_b, sb["Zb"])
                        sb["Zi_b"] = Zi_b
                    Zi_b = sb["Zi_b"]
                    zp_ps = psO.tile([128, 2], f32, tag="o", name="zp_ps")
                    for p01 in range(2):
                        nc.tensor.matmul(
                            zp_ps[:, p01:p01 + 1], perm8[:, p01, :], Zi_b,
                            start=True, stop=True,
                        )
                    pT = work.tile([128, 4, 128], f16, tag="pT", bufs=3)
                    nc.vector.tensor_copy(pT.rearrange("p w c -> p (w c)"), pt_ps)
                    ZiPs = work.tile([128, 2], f32, tag="ZiPs", bufs=4)
                    nc.vector.tensor_copy(ZiPs, zp_ps)
                    sb["pT"] = pT
                    sb["ZiPs"] = ZiPs

                # ---- output phase, part 2: gating (uses orow readback) ----
                if pending is not None and len(pending) == 2 and b % SG == 4:
                    sg, orow = pending
                    god = oph.tile([128, TOTAL], f16, tag="god")
                    nc.vector.tensor_mul(god, orow, sig_g[:, sg, :])
                    pending = (sg, orow, god)

                # ---- stage C: block b-3 ----
                if 0 <= b - 3 < NBLK:
                    bb = b - 3
                    sc = st.pop(bb)
                    vg = gath_tiles[bb // 2][1]
                    pT_r = sc["pT"].rearrange(
                        "p w (pp g h a) -> p w pp g h a", pp=2, g=2, h=H, a=4
                    )
                    av0 = psAV.tile([128, 512], f32, tag="av", name="av0")
                    av1 = psAV.tile([128, 512], f32, tag="av", name="av1")
                    avs = (av0, av1)
                    for p01 in range(2):
                        for asub in range(4):
                            s0 = 16 * (bb % 2) + 8 * p01 + asub
                            nc.tensor.matmul(
                                avs[p01][32 * asub:32 * asub + 16, :],
                                pT_r[:, asub, p01, :, :, asub],
                                vg[:, s0:s0 + 5:4, :],
                                start=True, stop=True,
                                tile_position=(0, 32 * asub),
                            )
                    if bb % SG == 0:
                        stage = work.tile([128, SG * 1024], f16, tag="stage")
                    nc.vector.tensor_scalar_mul(
                        stage[:, 1024 * (bb % SG):1024 * (bb % SG) + 512], av0,
                        sc["ZiPs"][:, 0:1],
                    )
                    nc.scalar.activation(
                        stage[:, 1024 * (bb % SG) + 512:1024 * (bb % SG + 1)],
                        av1, AF.Identity, scale=sc["ZiPs"][:, 1:2],
                    )

                    if bb % SG == SG - 1:
                        sg = bb // SG
                        st_r = stage.rearrange(
                            "p (blk p01 g01 h d) -> p blk p01 g01 h d",
                            blk=SG, p01=2, g01=2, h=H,
                        )
                        eng = (nc.sync, nc.scalar, nc.gpsimd)
                        for g01 in range(2):
                            for h in range(H):
                                eng[(g01 * H + h) % 3].dma_start(
                                    o_r[sg, :, :, :, g01, h, :],
                                    st_r[8 * g01 + h::32, :, :, g01, h, :],
                                )
                        orow = oph.tile([128, TOTAL], f16, tag="orow")
                        nc.gpsimd.dma_start(
                            orow, o_scr[128 * sg:128 * (sg + 1), :]
                        )
                        pending = (sg, orow)

                # ---- output phase, part 3: projection + store ----
                if pending is not None and len(pending) == 3 and b % SG == 6:
                    sg, orow, god = pending
                    godT = oph.tile([128, 2, 128], f16, tag="godT")
                    for hc in range(2):
                        gps = psO.tile([128, 128], f16, tag="o", name="gps")
                        nc.tensor.transpose(
                            gps, god[:, 128 * hc:128 * (hc + 1)], ident
                        )
                        nc.scalar.copy(godT[:, hc, :], gps)
                    for oc in range(2):
                        ops = psO.tile([128, 128], f32, tag="o", name="ops")
                        for hc in range(2):
                            nc.tensor.matmul(
                                ops, wo[:, hc, 128 * oc:128 * (oc + 1)],
                                godT[:, hc, :],
                                start=(hc == 0), stop=(hc == 1),
                            )
                        outs = oph.tile([128, 128], f32, tag="outs")
                        nc.scalar.activation(
                            outs, ops, AF.Identity, bias=bo_t[:, oc:oc + 1]
                        )
                        nc.scalar.dma_start(
                            out_t[128 * oc:128 * (oc + 1), 128 * sg:128 * (sg + 1)],
                            outs,
                        )
                    pending = None

            # drain the last supergroup's output phase
            if pending is not None:
                sg, orow = pending[0], pending[1]
                god = oph.tile([128, TOTAL], f16, tag="god")
                nc.vector.tensor_mul(god, orow, sig_g[:, sg, :])
                godT = oph.tile([128, 2, 128], f16, tag="godT")
                for hc in range(2):
                    gps = psO.tile([128, 128], f16, tag="o", name="gps")
                    nc.tensor.transpose(
                        gps, god[:, 128 * hc:128 * (hc + 1)], ident
                    )
                    nc.scalar.copy(godT[:, hc, :], gps)
                for oc in range(2):
                    ops = psO.tile([128, 128], f32, tag="o", name="ops")
                    for hc in range(2):
                        nc.tensor.matmul(
                            ops, wo[:, hc, 128 * oc:128 * (oc + 1)],
                            godT[:, hc, :],
                            start=(hc == 0), stop=(hc == 1),
                        )
                    outs = oph.tile([128, 128], f32, tag="outs")
                    nc.scalar.activation(
                        outs, ops, AF.Identity, bias=bo_t[:, oc:oc + 1]
                    )
                    nc.scalar.dma_start(
                        out_t[128 * oc:128 * (oc + 1), 128 * sg:128 * (sg + 1)],
                        outs,
                    )
    nc.finalize()
    return nc


def _host_prep(q, k, v, nlist, bias, Wq, Wk, Wv, Wg, bg, Wo, bo):
    """Build the 8 per-core input maps."""
    norm = D ** -0.5
    f32 = np.float32
    WqT = np.ascontiguousarray((Wq * norm).T.astype(np.float16))
    WgT = np.ascontiguousarray(Wg.T.astype(np.float16))
    WkT = np.ascontiguousarray(Wk.T.astype(np.float16))
    WvT = np.ascontiguousarray(Wv.T.astype(np.float16))
    WoTh = np.ascontiguousarray(Wo.T.astype(np.float16))
    bgr = np.ascontiguousarray(np.broadcast_to(bg.astype(f32), (128, TOTAL)))
    bo2 = np.ascontiguousarray(bo.astype(f32).reshape(2, 128).T)
    # perm[rz, p01, rav] = 1 iff rz = 64*p01 + 32*g01 + 4*h + asub
    # for rav = 32*asub + 8*g01 + h  (AV-psum row <- softmax row Z source)
    perm = np.zeros((128, 2, 128), np.float16)
    for p01 in range(2):
        for asub in range(4):
            for g01 in range(2):
                for h in range(H):
                    rav = 32 * asub + 8 * g01 + h
                    rz = 64 * p01 + 32 * g01 + 4 * h + asub
                    perm[rz, p01, rav] = 1.0

    in_maps = []
    for c in range(NCORES):
        f, chunk = c // CPF, c % CPF
        n0 = chunk * NLOC_C
        qc = q[f, n0:n0 + NLOC_C]                     # [512, 256]
        nl = nlist[f, n0:n0 + NLOC_C].astype(np.int16)  # [512, 128]
        # wrapped gather indices: per block b, t-th index at [16g + t%16, t//16]
        w = nl.reshape(NBLK, BLK * NNEI).reshape(NBLK, BLK * NNEI // 16, 16)
        w = np.transpose(w, (0, 2, 1)).reshape(NBLK, 16, -1)   # [b, 16, 128]
        w = np.concatenate([w] * 8, axis=1)                    # [b, 128, 128]
        idx_full = np.ascontiguousarray(
            np.transpose(w, (1, 0, 2)).reshape(128, NBLK * NNEI)
        )
        # bias: [8, 512, 128] -> [32 blocks, (g h asub), 128]
        bs = bias[f, :, n0:n0 + NLOC_C, :]
        from einops import rearrange as rr
        bias_cmp = rr(bs, "h (b g asub) i -> b (g h asub) i", b=NBLK, g=4, asub=4)
        bias_c = np.full((NBLK, 128, 4 * NNEI), -30000.0, np.float16)
        p_arange = np.arange(128)
        for asub in range(4):
            rows = p_arange[p_arange % 4 == asub]
            bias_c[:, rows, NNEI * asub:NNEI * (asub + 1)] = (
                bias_cmp[:, rows, :].astype(np.float16)
            )
        in_maps.append({
            "qT": np.ascontiguousarray(qc.T.astype(np.float16)),
            "kT": np.ascontiguousarray(k[f].T.astype(np.float16)),
            "vT": np.ascontiguousarray(v[f].T.astype(np.float16)),
            "WqT": WqT, "WgT": WgT, "WkT": WkT, "WvT": WvT, "WoTh": WoTh,
            "bgr": bgr, "bo2": bo2,
            "idx": idx_full, "bias_p": bias_c, "perm": perm,
        })
    return in_maps


def kernel(q, k, v, nlist, bias, Wq, Wk, Wv, Wg, bg, Wo, bo):
    from concourse.bass_utils import run_bass_kernel_spmd

    q = np.asarray(q, dtype=np.float32)
    k = np.asarray(k, dtype=np.float32)
    v = np.asarray(v, dtype=np.float32)
    bias = np.asarray(bias, dtype=np.float32)
    nlist_np = np.asarray(nlist)

    if "nc" not in _CACHE:
        _CACHE["nc"] = _build()
    nc = _CACHE["nc"]

    in_maps = _host_prep(
        q, k, v, nlist_np, bias,
        np.asarray(Wq, np.float32), np.asarray(Wk, np.float32),
        np.asarray(Wv, np.float32), np.asarray(Wg, np.float32),
        np.asarray(bg, np.float32), np.asarray(Wo, np.float32),
        np.asarray(bo, np.float32),
    )
    res = run_bass_kernel_spmd(nc, in_maps, core_ids=list(range(NCORES)))
    out = np.empty((NF, NLOC, TOTAL), dtype=np.float32)
    for c in range(NCORES):
        f, chunk = c // CPF, c % CPF
        n0 = chunk * NLOC_C
        out[f, n0:n0 + NLOC_C] = res.results[c]["out_t"].T
    return out


# revision 47
# speedup vs baseline: 1.0686x; 1.0153x over previous
"""Trainium2 Bass kernel for local (neighbor-list) multi-head attention.

Sharding: 8 cores = 2 frames x 4 atom-chunks (512 local atoms per core).
Per core: project k rows to SBUF (rank-striped) and v rows to DRAM in
fp16, DMA-row-gather neighbors (dma_gather; K from SBUF via transpose
mode, V from HBM; prefetched one block-pair ahead), per-block batched QK
(M=32 block-diag stationaries), softmax over a host-masked full-width
bias (unnormalized; 1/Z folded into the AV-psum evacuation via a
host-provided row-permutation matmul), PE-transpose, paired-atom AV
(M=16 stationaries), diagonal extraction via a DRAM bounce, gating +
output projection decoupled from the main loop.  The PE work is
software-pipelined two blocks deep (QK(b) | transpose(b-1) | AV(b-2))
so the tensor engine never head-blocks on the softmax chain.
"""

import numpy as np

NF, NLOC, NALL, NNEI = 2, 2048, 3072, 128
H, D = 8, 32
TOTAL = H * D          # 256
QDIM = 256
NCORES = 8
CPF = NCORES // NF     # 4 cores per frame
NLOC_C = NLOC // CPF   # 512 atoms per core
BLK = 16               # atoms per block
NBLK = NLOC_C // BLK   # 32
SG = 8                 # blocks per supergroup (=128 atoms)
NSG = NBLK // SG       # 4

_CACHE = {}


def _build():
    import concourse.bass as bass
    import concourse.mybir as mybir
    from concourse import bacc
    from concourse.tile import TileContext
    from concourse.masks import make_identity

    dt = mybir.dt
    f32, f16, i16 = dt.float32, dt.float16, dt.int16
    AF = mybir.ActivationFunctionType

    nc = bacc.Bacc(None, target_bir_lowering=False)

    # ---------------- external inputs (contents differ per core) ------------
    qT = nc.dram_tensor("qT", [QDIM, NLOC_C], f16, kind="ExternalInput")
    kT = nc.dram_tensor("kT", [QDIM, NALL], f16, kind="ExternalInput")
    vT = nc.dram_tensor("vT", [QDIM, NALL], f16, kind="ExternalInput")
    WqT = nc.dram_tensor("WqT", [QDIM, TOTAL], f16, kind="ExternalInput")
    WgT = nc.dram_tensor("WgT", [QDIM, TOTAL], f16, kind="ExternalInput")
    WkT = nc.dram_tensor("WkT", [QDIM, TOTAL], f16, kind="ExternalInput")
    WvT = nc.dram_tensor("WvT", [QDIM, TOTAL], f16, kind="ExternalInput")
    WoTh = nc.dram_tensor("WoTh", [TOTAL, QDIM], f16, kind="ExternalInput")
    bgr = nc.dram_tensor("bgr", [128, TOTAL], f32, kind="ExternalInput")
    bo2 = nc.dram_tensor("bo2", [128, 2], f32, kind="ExternalInput")
    idx = nc.dram_tensor("idx", [128, NBLK * NNEI], i16, kind="ExternalInput")
    bias_p = nc.dram_tensor("bias_p", [NBLK, 128, 4 * NNEI], f16, kind="ExternalInput")
    perm = nc.dram_tensor("perm", [128, 2, 128], f16, kind="ExternalInput")

    out_t = nc.dram_tensor("out_t", [TOTAL, NLOC_C], f32, kind="ExternalOutput")

    kT_r = kT.rearrange("(a p) n -> p a n", p=128)
    vT_r = vT.rearrange("(a p) n -> p a n", p=128)

    with TileContext(nc) as tc:
        with (
            tc.tile_pool(name="const", bufs=1) as const,
            tc.tile_pool(name="work", bufs=2) as work,
            tc.tile_pool(name="gath", bufs=2) as gath,
            tc.tile_pool(name="oph", bufs=1) as oph,
            tc.tile_pool(name="psQK", bufs=2, space="PSUM") as psQK,
            tc.tile_pool(name="psPT", bufs=2, space="PSUM") as psPT,
            tc.tile_pool(name="psAV", bufs=3, space="PSUM") as psAV,
            tc.tile_pool(name="psO", bufs=1, space="PSUM") as psO,
            tc.tile_pool(name="dram", bufs=1, space="DRAM") as dram,
        ):
            # ---------------- constants -------------------------------------
            ident = const.tile([128, 128], f16, tag="ident")
            make_identity(nc, ident)

            idx_tiles = {}

            def load_idx(sg):
                idx_t = work.tile([128, SG * NNEI], i16, tag="idx_t")
                nc.sync.dma_start(
                    idx_t, idx[:, SG * NNEI * sg:SG * NNEI * (sg + 1)]
                )
                idx_tiles[sg] = idx_t

            load_idx(0)
            wk = const.tile([128, 2, TOTAL], f16, tag="wk")
            nc.sync.dma_start(wk, WkT.rearrange("(a p) o -> p a o", p=128))
            wv = const.tile([128, 2, TOTAL], f16, tag="wv")
            nc.sync.dma_start(wv, WvT.rearrange("(a p) o -> p a o", p=128))

            # ---------------- K table (SBUF, rank-striped) -------------------
            khs = const.tile([128, NALL // 128, TOTAL], f16, tag="khs")
            for jc4 in range(NALL // 512):
                kTc = work.tile([128, 2, 512], f16, tag="kTc", bufs=1)
                nc.sync.dma_start(kTc, kT_r[:, :, 512 * jc4:512 * (jc4 + 1)])
                for j4 in range(4):
                    jc = 4 * jc4 + j4
                    ps = psQK.tile([128, TOTAL], f32, tag="qk", name="ps_k")
                    for cc in range(2):
                        nc.tensor.matmul(
                            ps, kTc[:, cc, 128 * j4:128 * (j4 + 1)], wk[:, cc, :],
                            start=(cc == 0), stop=(cc == 1),
                        )
                    nc.scalar.copy(khs[:, jc, :], ps)

            # ---------------- V table (DRAM rows, fp16) ----------------------
            vh_d = dram.tile([NALL, TOTAL], f16)
            for jc4 in range(NALL // 512):
                vTc = work.tile([128, 2, 512], f16, tag="kTc", bufs=1)
                nc.sync.dma_start(vTc, vT_r[:, :, 512 * jc4:512 * (jc4 + 1)])
                row16 = work.tile([128, 4, TOTAL], f16, tag="row16", bufs=1)
                for j4 in range(4):
                    ps = psQK.tile([128, TOTAL], f32, tag="qk", name="ps_v")
                    for cc in range(2):
                        nc.tensor.matmul(
                            ps, vTc[:, cc, 128 * j4:128 * (j4 + 1)], wv[:, cc, :],
                            start=(cc == 0), stop=(cc == 1),
                        )
                    nc.scalar.copy(row16[:, j4, :], ps)
                nc.scalar.dma_start(
                    vh_d[512 * jc4:512 * (jc4 + 1), :].rearrange(
                        "(c p) o -> p c o", p=128
                    ),
                    row16,
                )

            # ---------------- gather issue (prefetched one pair ahead) -------
            gath_tiles = {}

            def issue_gathers(pair):
                b0 = 2 * pair
                idx_sl = idx_tiles[b0 // SG][:, NNEI * (b0 % SG):NNEI * (b0 % SG + 2)]
                kgT = gath.tile([128, 2, 2 * BLK * NNEI], f16, tag="kgT")
                nc.gpsimd.dma_gather(
                    kgT, khs[:, :, :], idx_sl,
                    num_idxs=2 * BLK * NNEI, num_idxs_reg=2 * BLK * NNEI,
                    elem_size=TOTAL, transpose=True, queue_num=0,
                    single_packet=False,
                    sbuf_tokens_per_rank=128,
                    sbuf_free_dim_per_rank=2 * TOTAL,
                    sbuf_free_dim_pad_per_rank=0,
                    sbuf_byte_offset=0,
                )
                vg = gath.tile([128, 2 * BLK, TOTAL], f16, tag="vg", bufs=3)
                nc.gpsimd.dma_gather(
                    vg, vh_d[:, :], idx_sl,
                    num_idxs=2 * BLK * NNEI, num_idxs_reg=2 * BLK * NNEI,
                    elem_size=TOTAL, transpose=False, queue_num=0,
                    single_packet=False,
                )
                gath_tiles[pair] = (kgT, vg)

            issue_gathers(0)   # K side ready as soon as khs lands
            _PREFETCH = False

            # ---------------- per-supergroup bias ----------------------------
            bias_tiles = {}

            def load_bias(sg):
                bias_t = work.tile([128, SG, 4 * NNEI], f16, tag="bias_t", bufs=1)
                nc.sync.dma_start(
                    bias_t, bias_p[SG * sg:SG * (sg + 1)].rearrange("b p i -> p b i")
                )
                bias_tiles[sg] = bias_t

            load_bias(0)

            # ---------------- q-side ------------------------------------------
            wq = const.tile([128, 2, TOTAL], f16, tag="wq")
            nc.sync.dma_start(wq, WqT.rearrange("(a p) o -> p a o", p=128))
            wg = const.tile([128, 2, TOTAL], f16, tag="wg")
            nc.sync.dma_start(wg, WgT.rearrange("(a p) o -> p a o", p=128))
            wo = const.tile([128, 2, QDIM], f16, tag="wo")
            nc.sync.dma_start(wo, WoTh.rearrange("(a p) o -> p a o", p=128))
            bg_t = const.tile([128, TOTAL], f32, tag="bg_t")
            nc.sync.dma_start(bg_t, bgr[:, :])
            bo_t = const.tile([128, 2], f32, tag="bo_t")
            nc.sync.dma_start(bo_t, bo2[:, :])
            qT_t = const.tile([128, 2, NLOC_C], f16, tag="qT_t")
            nc.sync.dma_start(qT_t, qT.rearrange("(a p) n -> p a n", p=128))
            perm8 = const.tile([128, 2, 128], f16, tag="perm8")
            nc.sync.dma_start(perm8, perm[:, :, :])



            # qhT (fp16, [hd_chunk][128, NLOC_C])
            qhT = const.tile([128, 2, NLOC_C], f16, tag="qhT")
            for hc in range(2):
                ps = psQK.tile([128, NLOC_C], f32, tag="qk", name="ps_qh")
                for cc in range(2):
                    nc.tensor.matmul(
                        ps, wq[:, cc, 128 * hc:128 * (hc + 1)], qT_t[:, cc, :],
                        start=(cc == 0), stop=(cc == 1),
                    )
                nc.scalar.copy(qhT[:, hc, :], ps)

            # sigmoid(g) rows: [n_chunk][128, 256]
            sig_g = const.tile([128, 4, TOTAL], f32, tag="sig_g")
            for ncnk in range(4):
                ps = psQK.tile([128, TOTAL], f32, tag="qk", name="ps_g")
                for cc in range(2):
                    nc.tensor.matmul(
                        ps, qT_t[:, cc, 128 * ncnk:128 * (ncnk + 1)], wg[:, cc, :],
                        start=(cc == 0), stop=(cc == 1),
                    )
                gtmp = work.tile([128, TOTAL], f32, tag="gtmp", bufs=1)
                nc.vector.tensor_add(gtmp, ps, bg_t)
                nc.scalar.activation(sig_g[:, ncnk, :], gtmp, AF.Sigmoid)

            # qblk: block-diagonal stationaries [128, ch, NBLK*4 groups * 32]
            qblk = const.tile([128, 2, (NLOC_C // 4) * 32], f16, tag="qblk")
            nc.gpsimd.memset(qblk, 0.0)
            for ch in range(2):
                for qq in range(4):
                    h = 4 * ch + qq
                    dst = qblk[32 * qq:32 * (qq + 1), ch, :].rearrange(
                        "p (G c) -> p G c", c=32
                    )[:, :, 4 * h:4 * h + 4]
                    src = qhT[32 * qq:32 * (qq + 1), ch, :].rearrange(
                        "p (G a) -> p G a", a=4
                    )
                    nc.vector.tensor_copy(dst, src)

            # staging tensors
            o_scr = dram.tile([NLOC_C, TOTAL], f16)
            o_r = o_scr.rearrange(
                "(sg blk p01 g01 asub) (h d) -> sg asub blk p01 g01 h d",
                sg=NSG, blk=SG, p01=2, g01=2, asub=4, h=H,
            )

            # ---------------- software-pipelined main loop --------------------
            # stage A (block b):   QK + softmax chain + 1/Z recip
            # stage B (block b-1): P transposes + 1/Z permutation matmuls
            # stage C (block b-2): AV + scaled evac (+ extract cadence)
            st = {}            # per-block tiles
            stage = None
            pending = None     # (sg, orow) or (sg, orow, god)
            for it in range(NBLK + 3):
                b = it
                if b < NBLK:
                    if b % 2 == 0:
                        if b > 0:
                            issue_gathers(b // 2)
                        if b % SG == 0 and b + SG < NBLK:
                            load_bias(b // SG + 1)
                            load_idx(b // SG + 1)
                    kgT = gath_tiles[b // 2][0]
                    qk = psQK.tile([128, 4 * NNEI], f32, tag="qk", name="qk")
                    for g in range(4):
                        for cc in range(2):
                            nc.tensor.matmul(
                                qk[32 * g:32 * (g + 1), :],
                                qblk[:, cc, 32 * (4 * b + g):32 * (4 * b + g + 1)],
                                kgT[:, cc, 512 * (4 * (b % 2) + g):512 * (4 * (b % 2) + g + 1)],
                                start=(cc == 0), stop=(cc == 1),
                                tile_position=(0, 32 * g),
                            )
                    # 1/Z for the previous block: first in the DVE queue this
                    # iteration so the stage-B permutation matmuls never stall
                    if b - 1 >= 0:
                        Zi_b = work.tile([128, 1], f16, tag="Zi_b", bufs=3)
                        with nc.allow_low_precision(reason="1/Z feeds fp16 p"):
                            nc.vector.reciprocal(Zi_b, st[b - 1]["Zb"])
                        st[b - 1]["Zi_b"] = Zi_b
                    s_t = work.tile([128, 4 * NNEI], f32, tag="s_t", bufs=2)
                    nc.vector.tensor_add(s_t, qk, bias_tiles[b // SG][:, b % SG, :])
                    m_t = work.tile([128, 1], f32, tag="m_t", bufs=3)
                    nc.vector.reduce_max(
                        m_t, s_t, axis=mybir.AxisListType.X, negate=True
                    )
                    p_t = work.tile([128, 4 * NNEI], f16, tag="p_t", bufs=3)
                    Zb = work.tile([128, 1], f32, tag="Zb", bufs=3)
                    nc.scalar.activation(
                        p_t, s_t, AF.Exp, bias=m_t, scale=1.0, accum_out=Zb,
                    )
                    st[b] = {"p_t": p_t, "Zb": Zb}

                # ---- stage B: block b-2 ----
                if 0 <= b - 2 < NBLK:
                    sb = st[b - 2]
                    pt_ps = psPT.tile([128, 4 * NNEI], f16, tag="pt")
                    for j in range(4):
                        nc.tensor.transpose(
                            pt_ps[:, 128 * j:128 * (j + 1)],
                            sb["p_t"][:, 128 * j:128 * (j + 1)], ident,
                        )
                    if "Zi_b" not in sb:   # last block: stage A already ended
                        Zi_b = work.tile([128, 1], f16, tag="Zi_b", bufs=3)
                        with nc.allow_low_precision(reason="1/Z feeds fp16 p"):
                            nc.vector.reciprocal(Zi_b, sb["Zb"])
                        sb["Zi_b"] = Zi_b
                    Zi_b = sb["Zi_b"]
                    zp_ps = psO.tile([128, 2], f32, tag="o", name="zp_ps")
                    for p01 in range(2):
                        nc.tensor.matmul(
                            zp_ps[:, p01:p01 + 1], perm8[:, p01, :], Zi_b,
                            start=True, stop=True,
                        )
                    pT = work.tile([128, 4, 128], f16, tag="pT", bufs=3)
                    nc.vector.tensor_copy(pT.rearrange("p w c -> p (w c)"), pt_ps)
                    ZiPs = work.tile([128, 2], f32, tag="ZiPs", bufs=3)
                    nc.vector.tensor_copy(ZiPs, zp_ps)
                    sb["pT"] = pT
                    sb["ZiPs"] = ZiPs

                # ---- output phase, part 2: gating (uses orow readback) ----
                if pending is not None and len(pending) == 2 and b % SG == 5:
                    sg, orow = pending
                    god = oph.tile([128, TOTAL], f16, tag="god")
                    nc.vector.tensor_mul(god, orow, sig_g[:, sg, :])
                    pending = (sg, orow, god)

                # ---- stage C: block b-3 ----
                if 0 <= b - 3 < NBLK:
                    bb = b - 3
                    sc = st.pop(bb)
                    vg = gath_tiles[bb // 2][1]
                    pT_r = sc["pT"].rearrange(
                        "p w (pp g h a) -> p w pp g h a", pp=2, g=2, h=H, a=4
                    )
                    av0 = psAV.tile([128, 512], f32, tag="av", name="av0")
                    av1 = psAV.tile([128, 512], f32, tag="av", name="av1")
                    avs = (av0, av1)
                    for p01 in range(2):
                        for asub in range(4):
                            s0 = 16 * (bb % 2) + 8 * p01 + asub
                            nc.tensor.matmul(
                                avs[p01][32 * asub:32 * asub + 16, :],
                                pT_r[:, asub, p01, :, :, asub],
                                vg[:, s0:s0 + 5:4, :],
                                start=True, stop=True,
                                tile_position=(0, 32 * asub),
                            )
                    if bb % SG == 0:
                        stage = work.tile([128, SG * 1024], f16, tag="stage")
                    nc.vector.tensor_scalar_mul(
                        stage[:, 1024 * (bb % SG):1024 * (bb % SG) + 512], av0,
                        sc["ZiPs"][:, 0:1],
                    )
                    nc.scalar.activation(
                        stage[:, 1024 * (bb % SG) + 512:1024 * (bb % SG + 1)],
                        av1, AF.Identity, scale=sc["ZiPs"][:, 1:2],
                    )

                    if bb % SG == SG - 1:
                        sg = bb // SG
                        st_r = stage.rearrange(
                            "p (blk p01 g01 h d) -> p blk p01 g01 h d",
                            blk=SG, p01=2, g01=2, h=H,
                        )
                        eng = (nc.sync, nc.scalar, nc.gpsimd)
                        for g01 in range(2):
                            for h in range(H):
                                eng[(g01 * H + h) % 3].dma_start(
                                    o_r[sg, :, :, :, g01, h, :],
                                    st_r[8 * g01 + h::32, :, :, g01, h, :],
                                )
                        orow = oph.tile([128, TOTAL], f16, tag="orow")
                        nc.gpsimd.dma_start(
                            orow, o_scr[128 * sg:128 * (sg + 1), :]
                        )
                        pending = (sg, orow)

                # ---- output phase, part 3: projection + store ----
                if pending is not None and len(pending) == 3 and b % SG == 7:
                    sg, orow, god = pending
                    godT = oph.tile([128, 2, 128], f16, tag="godT")
                    for hc in range(2):
                        gps = psO.tile([128, 128], f16, tag="o", name="gps")
                        nc.tensor.transpose(
                            gps, god[:, 128 * hc:128 * (hc + 1)], ident
                        )
                        nc.scalar.copy(godT[:, hc, :], gps)
                    for oc in range(2):
                        ops = psO.tile([128, 128], f32, tag="o", name="ops")
                        for hc in range(2):
                            nc.tensor.matmul(
                                ops, wo[:, hc, 128 * oc:128 * (oc + 1)],
                                godT[:, hc, :],
                                start=(hc == 0), stop=(hc == 1),
                            )
                        outs = oph.tile([128, 128], f32, tag="outs")
                        nc.scalar.activation(
                            outs, ops, AF.Identity, bias=bo_t[:, oc:oc + 1]
                        )
                        nc.scalar.dma_start(
                            out_t[128 * oc:128 * (oc + 1), 128 * sg:128 * (sg + 1)],
                            outs,
                        )
                    pending = None

            # drain the last supergroup's output phase
            if pending is not None:
                sg, orow = pending[0], pending[1]
                god = oph.tile([128, TOTAL], f16, tag="god")
                nc.vector.tensor_mul(god, orow, sig_g[:, sg, :])
                godT = oph.tile([128, 2, 128], f16, tag="godT")
                for hc in range(2):
                    gps = psO.tile([128, 128], f16, tag="o", name="gps")
                    nc.tensor.transpose(
                        gps, god[:, 128 * hc:128 * (hc + 1)], ident
                    )
                    nc.scalar.copy(godT[:, hc, :], gps)
                for oc in range(2):
                    ops = psO.tile([128, 128], f32, tag="o", name="ops")
                    for hc in range(2):
                        nc.tensor.matmul(
                            ops, wo[:, hc, 128 * oc:128 * (oc + 1)],
                            godT[:, hc, :],
                            start=(hc == 0), stop=(hc == 1),
                        )
                    outs = oph.tile([128, 128], f32, tag="outs")
                    nc.scalar.activation(
                        outs, ops, AF.Identity, bias=bo_t[:, oc:oc + 1]
                    )
                    nc.scalar.dma_start(
                        out_t[128 * oc:128 * (oc + 1), 128 * sg:128 * (sg + 1)],
                        outs,
                    )
    nc.finalize()
    return nc


def _host_prep(q, k, v, nlist, bias, Wq, Wk, Wv, Wg, bg, Wo, bo):
    """Build the 8 per-core input maps."""
    norm = D ** -0.5
    f32 = np.float32
    WqT = np.ascontiguousarray((Wq * norm).T.astype(np.float16))
    WgT = np.ascontiguousarray(Wg.T.astype(np.float16))
    WkT = np.ascontiguousarray(Wk.T.astype(np.float16))
    WvT = np.ascontiguousarray(Wv.T.astype(np.float16))
    WoTh = np.ascontiguousarray(Wo.T.astype(np.float16))
    bgr = np.ascontiguousarray(np.broadcast_to(bg.astype(f32), (128, TOTAL)))
    bo2 = np.ascontiguousarray(bo.astype(f32).reshape(2, 128).T)
    # perm[rz, p01, rav] = 1 iff rz = 64*p01 + 32*g01 + 4*h + asub
    # for rav = 32*asub + 8*g01 + h  (AV-psum row <- softmax row Z source)
    perm = np.zeros((128, 2, 128), np.float16)
    for p01 in range(2):
        for asub in range(4):
            for g01 in range(2):
                for h in range(H):
                    rav = 32 * asub + 8 * g01 + h
                    rz = 64 * p01 + 32 * g01 + 4 * h + asub
                    perm[rz, p01, rav] = 1.0

    in_maps = []
    for c in range(NCORES):
        f, chunk = c // CPF, c % CPF
        n0 = chunk * NLOC_C
        qc = q[f, n0:n0 + NLOC_C]                     # [512, 256]
        nl = nlist[f, n0:n0 + NLOC_C].astype(np.int16)  # [512, 128]
        # wrapped gather indices: per block b, t-th index at [16g + t%16, t//16]
        w = nl.reshape(NBLK, BLK * NNEI).reshape(NBLK, BLK * NNEI // 16, 16)
        w = np.transpose(w, (0, 2, 1)).reshape(NBLK, 16, -1)   # [b, 16, 128]
        w = np.concatenate([w] * 8, axis=1)                    # [b, 128, 128]
        idx_full = np.ascontiguousarray(
            np.transpose(w, (1, 0, 2)).reshape(128, NBLK * NNEI)
        )
        # bias: [8, 512, 128] -> [32 blocks, (g h asub), 128]
        bs = bias[f, :, n0:n0 + NLOC_C, :]
        from einops import rearrange as rr
        bias_cmp = rr(bs, "h (b g asub) i -> b (g h asub) i", b=NBLK, g=4, asub=4)
        bias_c = np.full((NBLK, 128, 4 * NNEI), -30000.0, np.float16)
        p_arange = np.arange(128)
        for asub in range(4):
            rows = p_arange[p_arange % 4 == asub]
            bias_c[:, rows, NNEI * asub:NNEI * (asub + 1)] = (
                bias_cmp[:, rows, :].astype(np.float16)
            )
        in_maps.append({
            "qT": np.ascontiguousarray(qc.T.astype(np.float16)),
            "kT": np.ascontiguousarray(k[f].T.astype(np.float16)),
            "vT": np.ascontiguousarray(v[f].T.astype(np.float16)),
            "WqT": WqT, "WgT": WgT, "WkT": WkT, "WvT": WvT, "WoTh": WoTh,
            "bgr": bgr, "bo2": bo2,
            "idx": idx_full, "bias_p": bias_c, "perm": perm,
        })
    return in_maps


def kernel(q, k, v, nlist, bias, Wq, Wk, Wv, Wg, bg, Wo, bo):
    from concourse.bass_utils import run_bass_kernel_spmd

    q = np.asarray(q, dtype=np.float32)
    k = np.asarray(k, dtype=np.float32)
    v = np.asarray(v, dtype=np.float32)
    bias = np.asarray(bias, dtype=np.float32)
    nlist_np = np.asarray(nlist)

    if "nc" not in _CACHE:
        _CACHE["nc"] = _build()
    nc = _CACHE["nc"]

    in_maps = _host_prep(
        q, k, v, nlist_np, bias,
        np.asarray(Wq, np.float32), np.asarray(Wk, np.float32),
        np.asarray(Wv, np.float32), np.asarray(Wg, np.float32),
        np.asarray(bg, np.float32), np.asarray(Wo, np.float32),
        np.asarray(bo, np.float32),
    )
    res = run_bass_kernel_spmd(nc, in_maps, core_ids=list(range(NCORES)))
    out = np.empty((NF, NLOC, TOTAL), dtype=np.float32)
    for c in range(NCORES):
        f, chunk = c // CPF, c % CPF
        n0 = chunk * NLOC_C
        out[f, n0:n0 + NLOC_C] = res.results[c]["out_t"].T
    return out


# revision 52
# speedup vs baseline: 1.1392x; 1.0661x over previous
"""Trainium2 Bass kernel for local (neighbor-list) multi-head attention.

Sharding: 8 cores = 2 frames x 4 atom-chunks (512 local atoms per core).
Per core: project k rows to SBUF (rank-striped) and v rows to DRAM in
fp16, DMA-row-gather neighbors (dma_gather; K from SBUF via transpose
mode, V from HBM; prefetched one block-pair ahead), per-block batched QK
(M=32 block-diag stationaries), softmax over a host-masked full-width
bias (unnormalized; 1/Z folded into the AV-psum evacuation via a
host-provided row-permutation matmul), PE-transpose, paired-atom AV
(M=16 stationaries), diagonal extraction via a DRAM bounce, gating +
output projection decoupled from the main loop.  The PE work is
software-pipelined two blocks deep (QK(b) | transpose(b-1) | AV(b-2))
so the tensor engine never head-blocks on the softmax chain.
"""

import numpy as np

NF, NLOC, NALL, NNEI = 2, 2048, 3072, 128
H, D = 8, 32
TOTAL = H * D          # 256
QDIM = 256
NCORES = 8
CPF = NCORES // NF     # 4 cores per frame
NLOC_C = NLOC // CPF   # 512 atoms per core
BLK = 16               # atoms per block
NBLK = NLOC_C // BLK   # 32
SG = 8                 # blocks per supergroup (=128 atoms)
NSG = NBLK // SG       # 4

_CACHE = {}


def _build():
    import concourse.bass as bass
    import concourse.mybir as mybir
    from concourse import bacc
    from concourse.tile import TileContext
    from concourse.masks import make_identity

    dt = mybir.dt
    f32, f16, i16 = dt.float32, dt.float16, dt.int16
    AF = mybir.ActivationFunctionType

    nc = bacc.Bacc(None, target_bir_lowering=False)

    # ---------------- external inputs (contents differ per core) ------------
    qT = nc.dram_tensor("qT", [QDIM, NLOC_C], f16, kind="ExternalInput")
    kT = nc.dram_tensor("kT", [QDIM, NALL], f16, kind="ExternalInput")
    vT = nc.dram_tensor("vT", [QDIM, NALL], f16, kind="ExternalInput")
    WqT = nc.dram_tensor("WqT", [QDIM, TOTAL], f16, kind="ExternalInput")
    WgT = nc.dram_tensor("WgT", [QDIM, TOTAL], f16, kind="ExternalInput")
    WkT = nc.dram_tensor("WkT", [QDIM, TOTAL], f16, kind="ExternalInput")
    WvT = nc.dram_tensor("WvT", [QDIM, TOTAL], f16, kind="ExternalInput")
    WoTh = nc.dram_tensor("WoTh", [TOTAL, QDIM], f16, kind="ExternalInput")
    bgr = nc.dram_tensor("bgr", [128, TOTAL], f32, kind="ExternalInput")
    bo2 = nc.dram_tensor("bo2", [128, 2], f32, kind="ExternalInput")
    idx = nc.dram_tensor("idx", [128, NBLK * NNEI], i16, kind="ExternalInput")
    bias_p = nc.dram_tensor("bias_p", [NBLK, 128, 4 * NNEI], f16, kind="ExternalInput")
    perm = nc.dram_tensor("perm", [128, 2, 128], f16, kind="ExternalInput")

    out_t = nc.dram_tensor("out_t", [TOTAL, NLOC_C], f32, kind="ExternalOutput")

    kT_r = kT.rearrange("(a p) n -> p a n", p=128)
    vT_r = vT.rearrange("(a p) n -> p a n", p=128)

    with TileContext(nc) as tc:
        with (
            tc.tile_pool(name="const", bufs=1) as const,
            tc.tile_pool(name="work", bufs=2) as work,
            tc.tile_pool(name="gath", bufs=2) as gath,
            tc.tile_pool(name="oph", bufs=1) as oph,
            tc.tile_pool(name="psQK", bufs=2, space="PSUM") as psQK,
            tc.tile_pool(name="psPT", bufs=2, space="PSUM") as psPT,
            tc.tile_pool(name="psAV", bufs=3, space="PSUM") as psAV,
            tc.tile_pool(name="psO", bufs=1, space="PSUM") as psO,
            tc.tile_pool(name="dram", bufs=1, space="DRAM") as dram,
        ):
            # ---------------- constants -------------------------------------
            ident = const.tile([128, 128], f16, tag="ident")
            make_identity(nc, ident)

            idx_tiles = {}

            def load_idx(sg):
                idx_t = work.tile([128, SG * NNEI], i16, tag="idx_t")
                nc.sync.dma_start(
                    idx_t, idx[:, SG * NNEI * sg:SG * NNEI * (sg + 1)]
                )
                idx_tiles[sg] = idx_t

            load_idx(0)
            wk = const.tile([128, 2, TOTAL], f16, tag="wk")
            nc.sync.dma_start(wk, WkT.rearrange("(a p) o -> p a o", p=128))
            wv = const.tile([128, 2, TOTAL], f16, tag="wv")
            nc.sync.dma_start(wv, WvT.rearrange("(a p) o -> p a o", p=128))

            # ---------------- K table (SBUF, rank-striped) -------------------
            khs = const.tile([128, NALL // 128, TOTAL], f16, tag="khs")
            for jc4 in range(NALL // 512):
                kTc = work.tile([128, 2, 512], f16, tag="kTc", bufs=1)
                nc.sync.dma_start(kTc, kT_r[:, :, 512 * jc4:512 * (jc4 + 1)])
                for j4 in range(4):
                    jc = 4 * jc4 + j4
                    ps = psQK.tile([128, TOTAL], f32, tag="qk", name="ps_k")
                    for cc in range(2):
                        nc.tensor.matmul(
                            ps, kTc[:, cc, 128 * j4:128 * (j4 + 1)], wk[:, cc, :],
                            start=(cc == 0), stop=(cc == 1),
                        )
                    nc.scalar.copy(khs[:, jc, :], ps)

            # ---------------- V table (DRAM rows, fp16) ----------------------
            vh_d = dram.tile([NALL, TOTAL], f16)
            for jc4 in range(NALL // 512):
                vTc = work.tile([128, 2, 512], f16, tag="kTc", bufs=1)
                nc.sync.dma_start(vTc, vT_r[:, :, 512 * jc4:512 * (jc4 + 1)])
                row16 = work.tile([128, 4, TOTAL], f16, tag="row16", bufs=1)
                for j4 in range(4):
                    ps = psQK.tile([128, TOTAL], f32, tag="qk", name="ps_v")
                    for cc in range(2):
                        nc.tensor.matmul(
                            ps, vTc[:, cc, 128 * j4:128 * (j4 + 1)], wv[:, cc, :],
                            start=(cc == 0), stop=(cc == 1),
                        )
                    nc.scalar.copy(row16[:, j4, :], ps)
                nc.scalar.dma_start(
                    vh_d[512 * jc4:512 * (jc4 + 1), :].rearrange(
                        "(c p) o -> p c o", p=128
                    ),
                    row16,
                )

            # ---------------- gather issue (prefetched one pair ahead) -------
            gath_tiles = {}

            def issue_gathers(pair):
                b0 = 2 * pair
                idx_sl = idx_tiles[b0 // SG][:, NNEI * (b0 % SG):NNEI * (b0 % SG + 2)]
                kgT = gath.tile([128, 2, 2 * BLK * NNEI], f16, tag="kgT")
                nc.gpsimd.dma_gather(
                    kgT, khs[:, :, :], idx_sl,
                    num_idxs=2 * BLK * NNEI, num_idxs_reg=2 * BLK * NNEI,
                    elem_size=TOTAL, transpose=True, queue_num=0,
                    single_packet=False,
                    sbuf_tokens_per_rank=128,
                    sbuf_free_dim_per_rank=2 * TOTAL,
                    sbuf_free_dim_pad_per_rank=0,
                    sbuf_byte_offset=0,
                )
                vg = gath.tile([128, 2 * BLK, TOTAL], f16, tag="vg", bufs=3)
                nc.gpsimd.dma_gather(
                    vg, vh_d[:, :], idx_sl,
                    num_idxs=2 * BLK * NNEI, num_idxs_reg=2 * BLK * NNEI,
                    elem_size=TOTAL, transpose=False, queue_num=0,
                    single_packet=False,
                )
                gath_tiles[pair] = (kgT, vg)

            issue_gathers(0)   # K side ready as soon as khs lands
            _PREFETCH = False

            # ---------------- per-supergroup bias ----------------------------
            bias_tiles = {}

            def load_bias(sg):
                bias_t = work.tile([128, SG, 4 * NNEI], f16, tag="bias_t", bufs=1)
                nc.sync.dma_start(
                    bias_t, bias_p[SG * sg:SG * (sg + 1)].rearrange("b p i -> p b i")
                )
                bias_tiles[sg] = bias_t

            load_bias(0)

            # ---------------- q-side ------------------------------------------
            wq = const.tile([128, 2, TOTAL], f16, tag="wq")
            nc.sync.dma_start(wq, WqT.rearrange("(a p) o -> p a o", p=128))
            wg = const.tile([128, 2, TOTAL], f16, tag="wg")
            nc.sync.dma_start(wg, WgT.rearrange("(a p) o -> p a o", p=128))
            wo = const.tile([128, 2, QDIM], f16, tag="wo")
            nc.sync.dma_start(wo, WoTh.rearrange("(a p) o -> p a o", p=128))
            bg_t = const.tile([128, TOTAL], f32, tag="bg_t")
            nc.sync.dma_start(bg_t, bgr[:, :])
            bo_t = const.tile([128, 2], f32, tag="bo_t")
            nc.sync.dma_start(bo_t, bo2[:, :])
            qT_t = const.tile([128, 2, NLOC_C], f16, tag="qT_t")
            nc.sync.dma_start(qT_t, qT.rearrange("(a p) n -> p a n", p=128))
            perm8 = const.tile([128, 2, 128], f16, tag="perm8")
            nc.sync.dma_start(perm8, perm[:, :, :])



            # qhT (fp16, [hd_chunk][128, NLOC_C])
            qhT = const.tile([128, 2, NLOC_C], f16, tag="qhT")
            for hc in range(2):
                ps = psQK.tile([128, NLOC_C], f32, tag="qk", name="ps_qh")
                for cc in range(2):
                    nc.tensor.matmul(
                        ps, wq[:, cc, 128 * hc:128 * (hc + 1)], qT_t[:, cc, :],
                        start=(cc == 0), stop=(cc == 1),
                    )
                nc.scalar.copy(qhT[:, hc, :], ps)

            # sigmoid(g) rows: [n_chunk][128, 256]
            sig_g = const.tile([128, 4, TOTAL], f32, tag="sig_g")
            for ncnk in range(4):
                ps = psQK.tile([128, TOTAL], f32, tag="qk", name="ps_g")
                for cc in range(2):
                    nc.tensor.matmul(
                        ps, qT_t[:, cc, 128 * ncnk:128 * (ncnk + 1)], wg[:, cc, :],
                        start=(cc == 0), stop=(cc == 1),
                    )
                gtmp = work.tile([128, TOTAL], f32, tag="gtmp", bufs=1)
                nc.vector.tensor_add(gtmp, ps, bg_t)
                nc.scalar.activation(sig_g[:, ncnk, :], gtmp, AF.Sigmoid)

            # qblk: block-diagonal stationaries [128, ch, NBLK*4 groups * 32]
            qblk = const.tile([128, 2, (NLOC_C // 4) * 32], f16, tag="qblk")
            nc.gpsimd.memset(qblk, 0.0)
            for ch in range(2):
                for qq in range(4):
                    h = 4 * ch + qq
                    dst = qblk[32 * qq:32 * (qq + 1), ch, :].rearrange(
                        "p (G c) -> p G c", c=32
                    )[:, :, 4 * h:4 * h + 4]
                    src = qhT[32 * qq:32 * (qq + 1), ch, :].rearrange(
                        "p (G a) -> p G a", a=4
                    )
                    nc.vector.tensor_copy(dst, src)

            # staging tensors
            o_scr = dram.tile([NLOC_C, TOTAL], f16)
            o_r = o_scr.rearrange(
                "(sg blk p01 g01 asub) (h d) -> sg asub blk p01 g01 h d",
                sg=NSG, blk=SG, p01=2, g01=2, asub=4, h=H,
            )

            # ---------------- software-pipelined main loop --------------------
            # stage A (block b):   QK + softmax chain + 1/Z recip
            # stage B (block b-1): P transposes + 1/Z permutation matmuls
            # stage C (block b-2): AV + scaled evac (+ extract cadence)
            st = {}            # per-block tiles
            stage = None
            pending = None     # (sg, orow) or (sg, orow, god)
            for it in range(NBLK + 3):
                b = it
                if b < NBLK:
                    if b % 2 == 0:
                        if b > 0:
                            issue_gathers(b // 2)
                        if b % SG == 0 and b + SG < NBLK:
                            load_bias(b // SG + 1)
                            load_idx(b // SG + 1)
                    kgT = gath_tiles[b // 2][0]
                    qk = psQK.tile([128, 4 * NNEI], f32, tag="qk", name="qk")
                    for g in range(4):
                        for cc in range(2):
                            nc.tensor.matmul(
                                qk[32 * g:32 * (g + 1), :],
                                qblk[:, cc, 32 * (4 * b + g):32 * (4 * b + g + 1)],
                                kgT[:, cc, 512 * (4 * (b % 2) + g):512 * (4 * (b % 2) + g + 1)],
                                start=(cc == 0), stop=(cc == 1),
                                tile_position=(0, 32 * g),
                            )
                    # 1/Z for the previous block: first in the DVE queue this
                    # iteration so the stage-B permutation matmuls never stall
                    if b - 1 >= 0:
                        Zi_b = work.tile([128, 1], f16, tag="Zi_b", bufs=3)
                        with nc.allow_low_precision(reason="1/Z feeds fp16 p"):
                            nc.vector.reciprocal(Zi_b, st[b - 1]["Zb"])
                        st[b - 1]["Zi_b"] = Zi_b
                    s_t = work.tile([128, 4 * NNEI], f32, tag="s_t", bufs=2)
                    nc.vector.tensor_add(s_t, qk, bias_tiles[b // SG][:, b % SG, :])
                    m_t = work.tile([128, 1], f32, tag="m_t", bufs=3)
                    nc.vector.reduce_max(
                        m_t, s_t, axis=mybir.AxisListType.X, negate=True
                    )
                    p_t = work.tile([128, 4 * NNEI], f16, tag="p_t", bufs=3)
                    Zb = work.tile([128, 1], f32, tag="Zb", bufs=3)
                    nc.scalar.activation(
                        p_t, s_t, AF.Exp, bias=m_t, scale=1.0, accum_out=Zb,
                    )
                    st[b] = {"p_t": p_t, "Zb": Zb}

                # ---- stage B: block b-2 ----
                if 0 <= b - 2 < NBLK:
                    sb = st[b - 2]
                    pt_ps = psPT.tile([128, 4 * NNEI], f16, tag="pt")
                    for j in range(4):
                        nc.tensor.transpose(
                            pt_ps[:, 128 * j:128 * (j + 1)],
                            sb["p_t"][:, 128 * j:128 * (j + 1)], ident,
                        )
                    if "Zi_b" not in sb:   # last block: stage A already ended
                        Zi_b = work.tile([128, 1], f16, tag="Zi_b", bufs=3)
                        with nc.allow_low_precision(reason="1/Z feeds fp16 p"):
                            nc.vector.reciprocal(Zi_b, sb["Zb"])
                        sb["Zi_b"] = Zi_b
                    Zi_b = sb["Zi_b"]
                    zp_ps = psO.tile([128, 2], f32, tag="o", name="zp_ps")
                    for p01 in range(2):
                        nc.tensor.matmul(
                            zp_ps[:, p01:p01 + 1], perm8[:, p01, :], Zi_b,
                            start=True, stop=True,
                        )
                    pT = work.tile([128, 4, 128], f16, tag="pT", bufs=3)
                    nc.vector.tensor_copy(pT.rearrange("p w c -> p (w c)"), pt_ps)
                    ZiPs = work.tile([128, 2], f32, tag="ZiPs", bufs=3)
                    nc.vector.tensor_copy(ZiPs, zp_ps)
                    sb["pT"] = pT
                    sb["ZiPs"] = ZiPs

                # ---- output phase, part 2: gating (uses orow readback) ----
                if pending is not None and len(pending) == 2 and b % SG == 5:
                    sg, orow = pending
                    god = oph.tile([128, TOTAL], f16, tag="god")
                    nc.vector.tensor_mul(god, orow, sig_g[:, sg, :])
                    pending = (sg, orow, god)

                # ---- stage C: block b-3 ----
                if 0 <= b - 3 < NBLK:
                    bb = b - 3
                    sc = st.pop(bb)
                    vg = gath_tiles[bb // 2][1]
                    pT_r = sc["pT"].rearrange(
                        "p w (pp g h a) -> p w pp g h a", pp=2, g=2, h=H, a=4
                    )
                    av0 = psAV.tile([128, 512], f32, tag="av", name="av0")
                    av1 = psAV.tile([128, 512], f32, tag="av", name="av1")
                    avs = (av0, av1)
                    for p01 in range(2):
                        for asub in range(4):
                            s0 = 16 * (bb % 2) + 8 * p01 + asub
                            nc.tensor.matmul(
                                avs[p01][32 * asub:32 * asub + 16, :],
                                pT_r[:, asub, p01, :, :, asub],
                                vg[:, s0:s0 + 5:4, :],
                                start=True, stop=True,
                                tile_position=(0, 32 * asub),
                            )
                    if bb % SG == 0:
                        stage = work.tile([128, SG * 1024], f16, tag="stage")
                    nc.vector.tensor_scalar_mul(
                        stage[:, 1024 * (bb % SG):1024 * (bb % SG) + 512], av0,
                        sc["ZiPs"][:, 0:1],
                    )
                    nc.scalar.activation(
                        stage[:, 1024 * (bb % SG) + 512:1024 * (bb % SG + 1)],
                        av1, AF.Identity, scale=sc["ZiPs"][:, 1:2],
                    )

                    if bb % SG == SG - 1:
                        sg = bb // SG
                        st_r = stage.rearrange(
                            "p (blk p01 g01 h d) -> p blk p01 g01 h d",
                            blk=SG, p01=2, g01=2, h=H,
                        )
                        eng = (nc.sync, nc.scalar, nc.gpsimd)
                        for g01 in range(2):
                            for h in range(H):
                                eng[(g01 * H + h) % 3].dma_start(
                                    o_r[sg, :, :, :, g01, h, :],
                                    st_r[8 * g01 + h::32, :, :, g01, h, :],
                                )
                        orow = oph.tile([128, TOTAL], f16, tag="orow")
                        nc.gpsimd.dma_start(
                            orow, o_scr[128 * sg:128 * (sg + 1), :]
                        )
                        pending = (sg, orow)

                # ---- output phase, part 3: projection + store ----
                if pending is not None and len(pending) == 3 and b % SG == 7:
                    sg, orow, god = pending
                    godT = oph.tile([128, 2, 128], f16, tag="godT")
                    for hc in range(2):
                        gps = psO.tile([128, 128], f16, tag="o", name="gps")
                        nc.tensor.transpose(
                            gps, god[:, 128 * hc:128 * (hc + 1)], ident
                        )
                        nc.scalar.copy(godT[:, hc, :], gps)
                    for oc in range(2):
                        ops = psO.tile([128, 128], f32, tag="o", name="ops")
                        for hc in range(2):
                            nc.tensor.matmul(
                                ops, wo[:, hc, 128 * oc:128 * (oc + 1)],
                                godT[:, hc, :],
                                start=(hc == 0), stop=(hc == 1),
                            )
                        outs = oph.tile([128, 128], f32, tag="outs")
                        nc.scalar.activation(
                            outs, ops, AF.Identity, bias=bo_t[:, oc:oc + 1]
                        )
                        nc.scalar.dma_start(
                            out_t[128 * oc:128 * (oc + 1), 128 * sg:128 * (sg + 1)],
                            outs,
                        )
                    pending = None

            # drain the last supergroup's output phase
            if pending is not None:
                sg, orow = pending[0], pending[1]
                god = oph.tile([128, TOTAL], f16, tag="god")
                nc.vector.tensor_mul(god, orow, sig_g[:, sg, :])
                godT = oph.tile([128, 2, 128], f16, tag="godT")
                for hc in range(2):
                    gps = psO.tile([128, 128], f16, tag="o", name="gps")
                    nc.tensor.transpose(
                        gps, god[:, 128 * hc:128 * (hc + 1)], ident
                    )
                    nc.scalar.copy(godT[:, hc, :], gps)
                for oc in range(2):
                    ops = psO.tile([128, 128], f32, tag="o", name="ops")
                    for hc in range(2):
                        nc.tensor.matmul(
                            ops, wo[:, hc, 128 * oc:128 * (oc + 1)],
                            godT[:, hc, :],
                            start=(hc == 0), stop=(hc == 1),
                        )
                    outs = oph.tile([128, 128], f32, tag="outs")
                    nc.scalar.activation(
                        outs, ops, AF.Identity, bias=bo_t[:, oc:oc + 1]
                    )
                    nc.scalar.dma_start(
                        out_t[128 * oc:128 * (oc + 1), 128 * sg:128 * (sg + 1)],
                        outs,
                    )
    nc.finalize()
    return nc


def _host_prep(q, k, v, nlist, bias, Wq, Wk, Wv, Wg, bg, Wo, bo):
    """Build the 8 per-core input maps."""
    norm = D ** -0.5
    f32 = np.float32
    WqT = np.ascontiguousarray((Wq * norm).T.astype(np.float16))
    WgT = np.ascontiguousarray(Wg.T.astype(np.float16))
    WkT = np.ascontiguousarray(Wk.T.astype(np.float16))
    WvT = np.ascontiguousarray(Wv.T.astype(np.float16))
    WoTh = np.ascontiguousarray(Wo.T.astype(np.float16))
    bgr = np.ascontiguousarray(np.broadcast_to(bg.astype(f32), (128, TOTAL)))
    bo2 = np.ascontiguousarray(bo.astype(f32).reshape(2, 128).T)
    # perm[rz, p01, rav] = 1 iff rz = 64*p01 + 32*g01 + 4*h + asub
    # for rav = 32*asub + 8*g01 + h  (AV-psum row <- softmax row Z source)
    perm = np.zeros((128, 2, 128), np.float16)
    for p01 in range(2):
        for asub in range(4):
            for g01 in range(2):
                for h in range(H):
                    rav = 32 * asub + 8 * g01 + h
                    rz = 64 * p01 + 32 * g01 + 4 * h + asub
                    perm[rz, p01, rav] = 1.0

    in_maps = []
    for c in range(NCORES):
        f, chunk = c // CPF, c % CPF
        n0 = chunk * NLOC_C
        qc = q[f, n0:n0 + NLOC_C]                     # [512, 256]
        nl = nlist[f, n0:n0 + NLOC_C].astype(np.int16)  # [512, 128]
        # wrapped gather indices: per block b, t-th index at [16g + t%16, t//16]
        w = nl.reshape(NBLK, BLK * NNEI).reshape(NBLK, BLK * NNEI // 16, 16)
        w = np.transpose(w, (0, 2, 1)).reshape(NBLK, 16, -1)   # [b, 16, 128]
        w = np.concatenate([w] * 8, axis=1)                    # [b, 128, 128]
        idx_full = np.ascontiguousarray(
            np.transpose(w, (1, 0, 2)).reshape(128, NBLK * NNEI)
        )
        # bias: [8, 512, 128] -> [32 blocks, (g h asub), 128]
        bs = bias[f, :, n0:n0 + NLOC_C, :]
        from einops import rearrange as rr
        bias_cmp = rr(bs, "h (b g asub) i -> b (g h asub) i", b=NBLK, g=4, asub=4)
        bias_c = np.full((NBLK, 128, 4 * NNEI), -30000.0, np.float16)
        p_arange = np.arange(128)
        for asub in range(4):
            rows = p_arange[p_arange % 4 == asub]
            bias_c[:, rows, NNEI * asub:NNEI * (asub + 1)] = (
                bias_cmp[:, rows, :].astype(np.float16)
            )
        in_maps.append({
            "qT": np.ascontiguousarray(qc.T.astype(np.float16)),
            "kT": np.ascontiguousarray(k[f].T.astype(np.float16)),
            "vT": np.ascontiguousarray(v[f].T.astype(np.float16)),
            "WqT": WqT, "WgT": WgT, "WkT": WkT, "WvT": WvT, "WoTh": WoTh,
            "bgr": bgr, "bo2": bo2,
            "idx": idx_full, "bias_p": bias_c, "perm": perm,
        })
    return in_maps


def kernel(q, k, v, nlist, bias, Wq, Wk, Wv, Wg, bg, Wo, bo):
    from concourse.bass_utils import run_bass_kernel_spmd

    q = np.asarray(q, dtype=np.float32)
    k = np.asarray(k, dtype=np.float32)
    v = np.asarray(v, dtype=np.float32)
    bias = np.asarray(bias, dtype=np.float32)
    nlist_np = np.asarray(nlist)

    if "nc" not in _CACHE:
        _CACHE["nc"] = _build()
    nc = _CACHE["nc"]

    in_maps = _host_prep(
        q, k, v, nlist_np, bias,
        np.asarray(Wq, np.float32), np.asarray(Wk, np.float32),
        np.asarray(Wv, np.float32), np.asarray(Wg, np.float32),
        np.asarray(bg, np.float32), np.asarray(Wo, np.float32),
        np.asarray(bo, np.float32),
    )
    res = run_bass_kernel_spmd(nc, in_maps, core_ids=list(range(NCORES)))
    out = np.empty((NF, NLOC, TOTAL), dtype=np.float32)
    for c in range(NCORES):
        f, chunk = c // CPF, c % CPF
        n0 = chunk * NLOC_C
        out[f, n0:n0 + NLOC_C] = res.results[c]["out_t"].T
    return out
